# revision 8
# baseline (speedup 1.0000x reference)
"""EntNet Trainium2 kernel (8-core data-parallel over batch).

Reference computation (shapes: B=64, S=128, L=32, D=100, M=20, V=50000):
  sents = (emb[tokens] * mult).sum(axis=2)            # [B,S,D]
  mem0 = broadcast(keys)                              # [B,M,D]
  per step t: gate = sigmoid(s.mem + s.keys); cand = prelu(mem@Uw.T + keys@Vw.T + s@Ww.T)
              mem = normalize(mem + cand*gate, axis=D)

Kernel strategy per core (8 batches/core, R = 8*20 = 160 (b,m) rows):
  - Augmented embedding table embaug = [emb | emb@keys.T | 0pad] (V x 128)
    gathered with ACCUMULATING indirect DMAs (compute_op=add): the 32-word
    sentence sum happens inside the DMA engines, so sentence vectors AND the
    key-gate logits kg = s.keys come out of the gather with zero PE work.
  - Gather geometry: 8 groups x 32 word-gathers of 128 sentences each; 4
    partial accumulators per group keep same-destination DMAs >=4 issue slots
    apart (no transfer-WAW stalls); partials summed on DVE.
  - Recurrence kept in scale-free form: with U unnormalized and
    rho = 1/||U|| per row, the update
        mem' = normalize(mem + cand * sigmoid(l))
    is exactly
        U' = (1 + exp(-l)) . U + Uw@U + (Vk + Ws_t) * n        (n = ||U||)
    Gate split: exp(-l) = exp(-rho * 1^T(U.s)) * exp(-kg), with exp(-kg)
    precomputed per 16-step group (kg rows extracted by a SBUF->SBUF DMA
    transpose-gather), so no key-term matmul and no n-scaled keys on the
    critical path.
  - All matmuls run as float32r (single-pass PE mode) instead of float32
    (which costs 2 half-rate passes + 2 LDWEIGHTS each).
"""

import numpy as np

B, S, L, D, M, V = 64, 128, 32, 100, 20, 50000
NCORES = 8
BL = B // NCORES            # 8 batches per core
NS = BL * S                 # 1024 sentences per core
R = BL * M                  # 160 (b, m) rows per core
NG = 8                      # gather groups (16 steps each)
GSTEP = S // NG             # 16 steps per group
NPART = 4                   # partial accumulators per group
EW = 128                    # augmented embedding row width
RESCALE = 8                 # renormalize U every RESCALE steps (f32 range)

_prog_cache = {}

_ENGINE_SEM = {"PE": "PE_", "DVE": "DVE_", "Activation": "Activation_",
               "Pool": "Pool_", "SP": "SP_"}


def _strip_redundant_self_waits(nc):
    """Legalize sync waits: walrus rejects >1 sync wait on most instruction
    structs. For any instruction carrying several, hoist all but one onto
    preceding single-wait NoOps on the same engine queue (in-order dispatch
    keeps semantics). The instruction keeps its OWN-engine wait if it has one
    (that wait guards an engine-pipelining RAW hazard and must gate execution,
    not just dispatch).
    """
    import concourse.mybir as mybir
    for fn in nc.m.functions:
        for blk in fn.blocks:
            i = 0
            while i < len(blk.instructions):
                inst = blk.instructions[i]
                si = inst.sync_info() if callable(inst.sync_info) else inst.sync_info
                if (si is not None and si.on_wait and len(si.on_wait) > 1
                        and inst.engine is not None):
                    waits = list(si.on_wait)
                    pref = _ENGINE_SEM.get(inst.engine.name)
                    keep_idx = None
                    for j, w in enumerate(waits):
                        if pref and w.ant_name.startswith(pref):
                            keep_idx = j
                            break
                    kept = [waits.pop(keep_idx)] if keep_idx is not None else []
                    noops = []
                    for w in waits:
                        nop = mybir.InstNoOp(
                            name=nc.get_next_instruction_name(), ins=[], outs=[])
                        nop.engine = inst.engine
                        nop.sync_info = mybir.SyncInfo(on_wait=[w], on_update=[])
                        nc.register_instruction(nop, overwrite=True)
                        noops.append(nop)
                    inst.sync_info = mybir.SyncInfo(
                        on_wait=kept, on_update=list(si.on_update))
                    blk.instructions[i:i] = noops
                    i += len(noops)
                i += 1


def _build_program(a_is_one: bool, mult_is_ones: bool, alpha: float,
                   n_steps: int = S, dump: bool = False):
    import concourse.bass as bass
    import concourse.tile as tile
    from concourse import mybir
    from contextlib import ExitStack

    assert mult_is_ones and a_is_one, "fast path only (host fallback otherwise)"

    f32 = mybir.dt.float32
    f32r = mybir.dt.float32r
    i32 = mybir.dt.int32
    AF = mybir.ActivationFunctionType
    OP = mybir.AluOpType

    nc = bass.Bass(trn_type="TRN2")

    # ---- DRAM I/O ----
    CW = 549
    tok_d = nc.dram_tensor("tok", [128, NG * L], i32, kind="ExternalInput").ap()
    emb_d = nc.dram_tensor("embaug", [V, EW], f32, kind="ExternalInput").ap()
    consts_d = nc.dram_tensor("consts", [128, CW], f32r, kind="ExternalInput").ap()
    out_d = nc.dram_tensor("memT", [D, R], f32, kind="ExternalOutput").ap()

    def r_(ap):
        # f32r = fp32 bits fed to the PE in single-pass (replicated) mode.
        # walrus's BIR verifier requires producers of f32r-matmul operands to
        # declare f32r-rounded output, so producer OUT aps are bitcast too.
        return ap.bitcast(f32r)

    def mm(out, lhsT, rhs, start=True, stop=True):
        nc.tensor.matmul(out=out, lhsT=r_(lhsT), rhs=r_(rhs),
                         start=start, stop=stop)

    def bcast_mid(ap_2d, n_mid):
        # [P, k] -> [P, n_mid, k] with stride-0 middle dim
        return bass.AP(ap_2d.tensor, ap_2d.offset,
                       [list(ap_2d.ap[0]), [0, n_mid], list(ap_2d.ap[1])])

    def bcast_last(ap_2d, n_last):
        # [P, k] -> [P, k, n_last] with stride-0 last dim
        return bass.AP(ap_2d.tensor, ap_2d.offset,
                       [list(ap_2d.ap[0]), list(ap_2d.ap[1]), [0, n_last]])

    def bcast_mid2(ap_2d, n1, n2):
        # [P, k] -> [P, n1, n2, k] with stride-0 middle dims
        return bass.AP(ap_2d.tensor, ap_2d.offset,
                       [list(ap_2d.ap[0]), [0, n1], [0, n2], list(ap_2d.ap[1])])

    def bcast_last3(ap_3d, n_last):
        # [P, a, b] -> [P, a, b, n_last] with stride-0 last dim
        return bass.AP(ap_3d.tensor, ap_3d.offset,
                       [list(ap_3d.ap[0]), list(ap_3d.ap[1]),
                        list(ap_3d.ap[2]), [0, n_last]])

    with tile.TileContext(nc) as tc, ExitStack() as ctx:
        const = ctx.enter_context(tc.tile_pool(name="const", bufs=1))
        work = ctx.enter_context(tc.tile_pool(name="work", bufs=2))
        ps_setup = ctx.enter_context(tc.tile_pool(name="ps_setup", bufs=1, space="PSUM"))
        ps_loop = ctx.enter_context(tc.tile_pool(name="ps_loop", bufs=1, space="PSUM"))

        # ---- load constants / weights ----
        tok_sb = const.tile([128, NG * L], i32)
        nc.gpsimd.dma_start(out=tok_sb[:], in_=tok_d)
        consts = const.tile([128, CW], f32)
        nc.sync.dma_start(out=r_(consts[:]), in_=consts_d)
        keysT = consts[0:D, 0:M]
        UwT = consts[0:D, 20:120]
        WwT = consts[0:D, 120:220]
        VwT = consts[0:D, 220:320]
        onesD = consts[0:D, 320:321]
        ones1 = consts[0:1, 321:421]
        ident = consts[0:128, 421:549]

        # ---- Vk = Vw @ keys^T (early; only needs weights) ----
        ps_vk = ps_setup.tile([D, M], f32, tag="psws", name="ps_vk")
        mm(ps_vk[:], VwT, keysT)
        Vk = const.tile([D, M], f32)
        nc.scalar.copy(out=Vk[:], in_=ps_vk[:])

        # ---- gather machinery ----
        # group g covers steps 16g..16g+15 (128 sentences, row p = 8*tg + b);
        # 32 word-gathers accumulate emb-sum and kg into 4 partial tiles.
        parts = [[const.tile([128, EW], f32, name=f"part{g}_{q}",
                             tag=f"part{g % 2}_{q}")
                  for q in range(NPART)] for g in range(NG)]
        sentsS = [const.tile([128, EW], f32, name=f"sentsS{g}", tag=f"sS{g % 2}")
                  for g in range(NG)]
        sentsD = [const.tile([D, 128], f32, name=f"sentsD{g}", tag=f"sD{g % 2}")
                  for g in range(NG)]
        Ws = [const.tile([D, 128], f32, name=f"ws{g}", tag=f"ws{g % 2}")
              for g in range(NG)]
        kgrow = [const.tile([1, GSTEP * R], f32, name=f"kgrow{g}",
                            tag=f"kg{g % 2}") for g in range(NG)]
        ekg = [const.tile([1, GSTEP * R], f32, name=f"ekg{g}",
                          tag=f"ekg{g % 2}") for g in range(NG)]
        vw = [const.tile([D, GSTEP * R], f32, name=f"vw{g}", tag=f"vw{g % 2}")
              for g in range(NG)]

        def emit_gather(g, l):
            q = l % NPART
            nc.gpsimd.indirect_dma_start(
                out=parts[g][q][:],
                out_offset=None,
                in_=emb_d,
                in_offset=bass.IndirectOffsetOnAxis(
                    ap=tok_sb[:, g * L + l:g * L + l + 1], axis=0),
                compute_op=(OP.bypass if l < NPART else OP.add),
            )

        def emit_postproc(g):
            # combine partials -> sentence-major [sents | kg]
            e01 = work.tile([128, EW], f32, tag="comb", name=f"e01_{g}")
            nc.vector.tensor_tensor(out=e01[:], in0=parts[g][0][:],
                                    in1=parts[g][1][:], op=OP.add)
            e23 = work.tile([128, EW], f32, tag="comb", name=f"e23_{g}")
            nc.vector.tensor_tensor(out=e23[:], in0=parts[g][2][:],
                                    in1=parts[g][3][:], op=OP.add)
            nc.vector.tensor_tensor(out=sentsS[g][:], in0=e01[:], in1=e23[:],
                                    op=OP.add)
            # sents -> D-major via PE transpose
            pstr = ps_setup.tile([D, 128], f32, tag="pstr", name=f"pstr{g}")
            nc.tensor.transpose(out=pstr[:], in_=sentsS[g][:, 0:D],
                                identity=ident)
            nc.scalar.copy(out=r_(sentsD[g][:]), in_=pstr[:])
            # Ws = Ww @ sents
            psws = ps_setup.tile([D, 128], f32, tag="psws", name=f"psws{g}")
            mm(psws[:], WwT, sentsD[g][:])
            nc.scalar.copy(out=Ws[g][:], in_=psws[:])
            # kg rows: [128 sent, 20] -> [1, 16*160] via SBUF->SBUF DMA
            nc.sync.dma_start(out=kgrow[g][:], in_=sentsS[g][:, D:D + M])

        def emit_chunk(g, c):
            # ekg chunk: exp(-kg) for steps 4c..4c+3 of group g
            sl = slice(4 * c * R, (4 * c + 4) * R)
            nc.scalar.activation(out=ekg[g][:, sl], in_=kgrow[g][:, sl],
                                 func=AF.Exp, scale=-1.0)
            # vw chunk: Vk + Ws broadcast, [D, 4*160] view [D, 4, 8, 20]
            vsl = vw[g][:, 4 * c * R:(4 * c + 4) * R]
            out = bass.AP(vsl.tensor, vsl.offset,
                          [list(vsl.ap[0]), [R, 4], [M, BL], [1, M]])
            ws_sl = Ws[g][:, 32 * c:32 * c + 32]
            ws3 = bass.AP(ws_sl.tensor, ws_sl.offset,
                          [list(ws_sl.ap[0]), [8, 4], [1, 8]])
            nc.vector.tensor_tensor(out=out, in0=bcast_mid2(Vk[:], 4, BL),
                                    in1=bcast_last3(ws3, M), op=OP.add)

        # ---- lead-in: group 0 gathers + postproc + chunks; group 1 streams
        # during window 0 ----
        for l in range(L):
            emit_gather(0, l)
        emit_postproc(0)
        for c in range(NPART):
            emit_chunk(0, c)

        # ---- initial state ----
        U = work.tile([D, R], f32, tag="U")
        nc.vector.tensor_copy(out=r_(U[:].rearrange("d (b m) -> d b m", m=M)),
                              in_=bcast_mid(keysT, BL))
        rho = None
        vkwsn = None

        # ---- recurrence ----
        for t in range(n_steps):
            g, k = t // GSTEP, t % GSTEP
            rs = (t % RESCALE == 0)           # U exactly normalized (rho=n=1)
            rescale_now = ((t + 1) % RESCALE == 0)

            # stream next group's gathers: 3/step over k=0..10
            gn = g + 1
            if gn < NG:
                if k <= 10:
                    for j in range(3 * k, min(3 * k + 3, L)):
                        emit_gather(gn, j)
                elif k == 11:
                    emit_postproc(gn)
                elif k in (12, 13, 14):
                    emit_chunk(gn, k - 12)
            if k == 0 and g >= 1:
                emit_chunk(g, 3)

            # cand (n-scaled): psA = Uw@U; candf = psA + vw*n
            psA = ps_loop.tile([D, R], f32, tag="psA")
            mm(psA[:], UwT, U[:])

            # gate reduce: psmg = 1^T (U . s_t)
            mgt = work.tile([D, BL, M], f32, tag="mgt")
            nc.vector.tensor_tensor(
                out=r_(mgt[:]),
                in0=U[:].rearrange("d (b m) -> d b m", m=M),
                in1=bcast_last(sentsD[g][:, BL * k:BL * (k + 1)], M),
                op=OP.mult)
            psmg = ps_loop.tile([1, R], f32, tag="psmg")
            mm(psmg[:], onesD, mgt[:].rearrange("d b m -> d (b m)"))

            if rs:
                l_ap = psmg[:]
            else:
                l_sb = work.tile([1, R], f32, tag="l")
                nc.vector.tensor_tensor(out=l_sb[:], in0=psmg[:], in1=rho[:],
                                        op=OP.mult)
                l_ap = l_sb[:]
            e1 = work.tile([1, R], f32, tag="e1")
            nc.scalar.activation(out=e1[:], in_=l_ap, func=AF.Exp, scale=-1.0)
            e_sb = work.tile([1, R], f32, tag="e")
            nc.vector.tensor_tensor(out=r_(e_sb[:]), in0=e1[:],
                                    in1=ekg[g][:, k * R:(k + 1) * R], op=OP.mult)

            # candf = psA + vw_t * n  (vkwsn from prev tail; raw vw when n=1)
            candf = work.tile([D, R], f32, tag="candf")
            if rs:
                nc.vector.tensor_tensor(out=candf[:], in0=psA[:],
                                        in1=vw[g][:, k * R:(k + 1) * R],
                                        op=OP.add)
            else:
                nc.vector.tensor_tensor(out=candf[:], in0=psA[:],
                                        in1=vkwsn, op=OP.add)

            # U' = (1 + e) . U + candf
            psbce = ps_loop.tile([D, R], f32, tag="psbce")
            mm(psbce[:], ones1, e_sb[:])
            V_sb = work.tile([D, R], f32, tag="V")
            nc.vector.scalar_tensor_tensor(out=V_sb[:], in0=psbce[:],
                                           scalar=1.0, in1=U[:],
                                           op0=OP.add, op1=OP.mult)
            U2 = work.tile([D, R], f32, tag="U")
            nc.vector.tensor_tensor(out=r_(U2[:]), in0=V_sb[:], in1=candf[:],
                                    op=OP.add)
            U = U2

            # norms: rho' = exp(-0.5 ln ss), n' = ss * rho'
            sq = work.tile([D, R], f32, tag="sq")
            nc.scalar.activation(out=r_(sq[:]), in_=U[:], func=AF.Square)
            psss = ps_loop.tile([1, R], f32, tag="psss")
            mm(psss[:], onesD, sq[:])
            lnss = work.tile([1, R], f32, tag="lnss")
            nc.scalar.activation(out=lnss[:], in_=psss[:], func=AF.Ln)
            rho2 = work.tile([1, R], f32, tag="rho")
            nc.scalar.activation(out=r_(rho2[:]), in_=lnss[:], func=AF.Exp,
                                 scale=-0.5)
            rho = rho2

            if rescale_now:
                # exact renormalization: U *= bc(rho); afterwards rho = n = 1
                psbcr = ps_loop.tile([D, R], f32, tag="psbcn", name=f"psbcr{t}")
                mm(psbcr[:], ones1, rho[:])
                U3 = work.tile([D, R], f32, tag="U")
                nc.vector.tensor_tensor(out=r_(U3[:]), in0=psbcr[:], in1=U[:],
                                        op=OP.mult)
                U = U3
            elif t < n_steps - 1:
                # n' for next step's vkwsn
                n_sb = work.tile([1, R], f32, tag="n")
                nc.vector.tensor_tensor(out=r_(n_sb[:]), in0=psss[:], in1=rho[:],
                                        op=OP.mult)
                psbcn = ps_loop.tile([D, R], f32, tag="psbcn")
                mm(psbcn[:], ones1, n_sb[:])
                tn = t + 1
                gg, kk = tn // GSTEP, tn % GSTEP
                vkwsn2 = work.tile([D, R], f32, tag="vkwsn")
                nc.vector.tensor_tensor(out=vkwsn2[:], in0=psbcn[:],
                                        in1=vw[gg][:, kk * R:(kk + 1) * R],
                                        op=OP.mult)
                vkwsn = vkwsn2[:]

        # ---- output ----
        if n_steps % RESCALE == 0:
            nc.sync.dma_start(out=out_d, in_=U[:])
        else:
            psbcr = ps_loop.tile([D, R], f32, tag="psbcn")
            mm(psbcr[:], ones1, rho[:])
            memT = work.tile([D, R], f32, tag="memT")
            nc.vector.tensor_tensor(out=memT[:], in0=psbcr[:], in1=U[:],
                                    op=OP.mult)
            nc.sync.dma_start(out=out_d, in_=memT[:])

    _strip_redundant_self_waits(nc)
    return nc


def _np_fallback(tokens, emb, keys, mult, Uw, Vw, Ww, prelu_a):
    tokens = np.asarray(tokens)
    emb = np.asarray(emb, np.float32)
    keys = np.asarray(keys, np.float32)
    mult = np.asarray(mult, np.float32)
    Uw, Vw, Ww = (np.asarray(x, np.float32) for x in (Uw, Vw, Ww))
    a = float(np.asarray(prelu_a).reshape(-1)[0])
    x = emb[tokens] * mult
    sents = x.sum(axis=2, dtype=np.float32)
    mem = np.broadcast_to(keys, (tokens.shape[0], M, D)).astype(np.float32).copy()
    Vk = keys @ Vw.T
    for t in range(tokens.shape[1]):
        s = sents[:, t, :]
        logits = np.einsum('bd,bmd->bm', s, mem) + s @ keys.T
        gate = 1.0 / (1.0 + np.exp(-logits))
        pre = mem @ Uw.T + Vk + (s @ Ww.T)[:, None, :]
        cand = np.where(pre >= 0, pre, a * pre)
        mask = np.where(logits == 0.0, 0.0, 1.0)
        mem = mem + cand * (gate * mask)[:, :, None]
        mem = mem / np.linalg.norm(mem, axis=2, keepdims=True)
    return mem.astype(np.float32)


def _stage_inputs(tokens, emb, keys, mult, Uw, Vw, Ww, prelu_a):
    """Host-side sharding/layout prep. Returns (in_maps, flags)."""
    tokens = np.asarray(tokens)
    emb = np.asarray(emb, dtype=np.float32)
    keys = np.asarray(keys, dtype=np.float32)
    mult = np.asarray(mult, dtype=np.float32)
    a = float(np.asarray(prelu_a).reshape(-1)[0])
    a_is_one = (a == 1.0)
    mult_is_ones = bool(np.all(mult == 1.0))

    embaug = np.zeros((V, EW), np.float32)
    embaug[:, 0:D] = emb
    embaug[:, D:D + M] = emb @ keys.T
    embaug = np.ascontiguousarray(embaug)

    CW = 549
    consts = np.zeros((128, CW), np.float32)
    consts[0:D, 0:M] = keys.T
    consts[0:D, 20:120] = np.asarray(Uw, np.float32).T        # lhsT for Uw@mem
    consts[0:D, 120:220] = np.asarray(Ww, np.float32).T
    consts[0:D, 220:320] = np.asarray(Vw, np.float32).T
    consts[0:D, 320:321] = 1.0                                # onesD
    consts[0:1, 321:421] = 1.0                                # ones1
    consts[0:128, 421:549] = np.eye(128, dtype=np.float32)    # ident (transpose)

    in_maps = []
    for c in range(NCORES):
        tc_ = tokens[c * BL:(c + 1) * BL]                     # [8, S, L]
        # tok_staged[p, 32g + l] = tokens[b=p%8, t=16g+p//8, l]
        arr = tc_.reshape(BL, NG, GSTEP, L).transpose(1, 2, 0, 3)  # [g,tg,b,l]
        arr = arr.reshape(NG, 128, L).transpose(1, 0, 2).reshape(128, NG * L)
        in_maps.append({"tok": np.ascontiguousarray(arr, np.int32),
                        "embaug": embaug, "consts": consts})
    return in_maps, a_is_one, mult_is_ones, a


def kernel(tokens, emb, keys, mult, Uw, Vw, Ww, prelu_a, _trace=False):
    from concourse.bass_utils import run_bass_kernel_spmd

    in_maps, a_is_one, mult_is_ones, a = _stage_inputs(
        tokens, emb, keys, mult, Uw, Vw, Ww, prelu_a)
    if not (a_is_one and mult_is_ones):
        return _np_fallback(tokens, emb, keys, mult, Uw, Vw, Ww, prelu_a)

    key = (a_is_one, mult_is_ones)
    if key not in _prog_cache:
        _prog_cache[key] = _build_program(a_is_one, mult_is_ones, a)
    nc = _prog_cache[key]

    res = run_bass_kernel_spmd(nc, in_maps, list(range(NCORES)), trace=_trace)
    out = np.empty((B, M, D), dtype=np.float32)
    for c in range(NCORES):
        memT = res.results[c]["memT"]                          # [D, R]
        out[c * BL:(c + 1) * BL] = memT.reshape(D, BL, M).transpose(1, 2, 0)
    kernel._last_results = res
    return out


# revision 9
# speedup vs baseline: 1.0141x; 1.0141x over previous
"""EntNet Trainium2 kernel (8-core data-parallel over batch).

Reference computation (shapes: B=64, S=128, L=32, D=100, M=20, V=50000):
  sents = (emb[tokens] * mult).sum(axis=2)            # [B,S,D]
  mem0 = broadcast(keys)                              # [B,M,D]
  per step t: gate = sigmoid(s.mem + s.keys); cand = prelu(mem@Uw.T + keys@Vw.T + s@Ww.T)
              mem = normalize(mem + cand*gate, axis=D)

Kernel strategy per core (8 batches/core, R = 8*20 = 160 (b,m) rows):
  - Augmented embedding table embaug = [emb | emb@keys.T | 0pad] (V x 128)
    gathered with ACCUMULATING indirect DMAs (compute_op=add): the 32-word
    sentence sum happens inside the DMA engines, so sentence vectors AND the
    key-gate logits kg = s.keys come out of the gather with zero PE work.
  - Gather geometry: 8 groups x 32 word-gathers of 128 sentences each; 4
    partial accumulators per group keep same-destination DMAs >=4 issue slots
    apart (no transfer-WAW stalls); partials summed on DVE.
  - Recurrence kept in scale-free form: with U unnormalized and
    rho = 1/||U|| per row, the update
        mem' = normalize(mem + cand * sigmoid(l))
    is exactly
        U' = (1 + exp(-l)) . U + Uw@U + (Vk + Ws_t) * n        (n = ||U||)
    Gate split: exp(-l) = exp(-rho * 1^T(U.s)) * exp(-kg), with exp(-kg)
    precomputed per 16-step group (kg rows extracted by a SBUF->SBUF DMA
    transpose-gather), so no key-term matmul and no n-scaled keys on the
    critical path.
  - All matmuls run as float32r (single-pass PE mode) instead of float32
    (which costs 2 half-rate passes + 2 LDWEIGHTS each).
"""

import numpy as np

B, S, L, D, M, V = 64, 128, 32, 100, 20, 50000
NCORES = 8
BL = B // NCORES            # 8 batches per core
NS = BL * S                 # 1024 sentences per core
R = BL * M                  # 160 (b, m) rows per core
NG = 8                      # gather groups (16 steps each)
GSTEP = S // NG             # 16 steps per group
NPART = 4                   # partial accumulators per group
EW = 128                    # augmented embedding row width
RESCALE = 8                 # renormalize U every RESCALE steps (f32 range)

_prog_cache = {}

_ENGINE_SEM = {"PE": "PE_", "DVE": "DVE_", "Activation": "Activation_",
               "Pool": "Pool_", "SP": "SP_"}


def _strip_redundant_self_waits(nc):
    """Legalize sync waits: walrus rejects >1 sync wait on most instruction
    structs. For any instruction carrying several, hoist all but one onto
    preceding single-wait NoOps on the same engine queue (in-order dispatch
    keeps semantics). The instruction keeps its OWN-engine wait if it has one
    (that wait guards an engine-pipelining RAW hazard and must gate execution,
    not just dispatch).
    """
    import concourse.mybir as mybir
    for fn in nc.m.functions:
        for blk in fn.blocks:
            i = 0
            while i < len(blk.instructions):
                inst = blk.instructions[i]
                si = inst.sync_info() if callable(inst.sync_info) else inst.sync_info
                if (si is not None and si.on_wait and len(si.on_wait) > 1
                        and inst.engine is not None):
                    waits = list(si.on_wait)
                    pref = _ENGINE_SEM.get(inst.engine.name)
                    keep_idx = None
                    for j, w in enumerate(waits):
                        if pref and w.ant_name.startswith(pref):
                            keep_idx = j
                            break
                    kept = [waits.pop(keep_idx)] if keep_idx is not None else []
                    noops = []
                    for w in waits:
                        nop = mybir.InstNoOp(
                            name=nc.get_next_instruction_name(), ins=[], outs=[])
                        nop.engine = inst.engine
                        nop.sync_info = mybir.SyncInfo(on_wait=[w], on_update=[])
                        nc.register_instruction(nop, overwrite=True)
                        noops.append(nop)
                    inst.sync_info = mybir.SyncInfo(
                        on_wait=kept, on_update=list(si.on_update))
                    blk.instructions[i:i] = noops
                    i += len(noops)
                i += 1


def _build_program(a_is_one: bool, mult_is_ones: bool, alpha: float,
                   n_steps: int = S, dump: bool = False):
    import concourse.bass as bass
    import concourse.tile as tile
    from concourse import mybir
    from contextlib import ExitStack

    assert mult_is_ones and a_is_one, "fast path only (host fallback otherwise)"

    f32 = mybir.dt.float32
    f32r = mybir.dt.float32r
    i32 = mybir.dt.int32
    AF = mybir.ActivationFunctionType
    OP = mybir.AluOpType

    nc = bass.Bass(trn_type="TRN2")

    # ---- DRAM I/O ----
    CW = 549
    tok_d = nc.dram_tensor("tok", [128, NG * L], i32, kind="ExternalInput").ap()
    emb_d = nc.dram_tensor("embaug", [V, EW], f32, kind="ExternalInput").ap()
    consts_d = nc.dram_tensor("consts", [128, CW], f32r, kind="ExternalInput").ap()
    out_d = nc.dram_tensor("memT", [D, R], f32, kind="ExternalOutput").ap()

    def r_(ap):
        # f32r = fp32 bits fed to the PE in single-pass (replicated) mode.
        # walrus's BIR verifier requires producers of f32r-matmul operands to
        # declare f32r-rounded output, so producer OUT aps are bitcast too.
        return ap.bitcast(f32r)

    def mm(out, lhsT, rhs, start=True, stop=True):
        nc.tensor.matmul(out=out, lhsT=r_(lhsT), rhs=r_(rhs),
                         start=start, stop=stop)

    def bcast_mid(ap_2d, n_mid):
        # [P, k] -> [P, n_mid, k] with stride-0 middle dim
        return bass.AP(ap_2d.tensor, ap_2d.offset,
                       [list(ap_2d.ap[0]), [0, n_mid], list(ap_2d.ap[1])])

    def bcast_last(ap_2d, n_last):
        # [P, k] -> [P, k, n_last] with stride-0 last dim
        return bass.AP(ap_2d.tensor, ap_2d.offset,
                       [list(ap_2d.ap[0]), list(ap_2d.ap[1]), [0, n_last]])

    def bcast_mid2(ap_2d, n1, n2):
        # [P, k] -> [P, n1, n2, k] with stride-0 middle dims
        return bass.AP(ap_2d.tensor, ap_2d.offset,
                       [list(ap_2d.ap[0]), [0, n1], [0, n2], list(ap_2d.ap[1])])

    def bcast_last3(ap_3d, n_last):
        # [P, a, b] -> [P, a, b, n_last] with stride-0 last dim
        return bass.AP(ap_3d.tensor, ap_3d.offset,
                       [list(ap_3d.ap[0]), list(ap_3d.ap[1]),
                        list(ap_3d.ap[2]), [0, n_last]])

    with tile.TileContext(nc) as tc, ExitStack() as ctx:
        const = ctx.enter_context(tc.tile_pool(name="const", bufs=1))
        work = ctx.enter_context(tc.tile_pool(name="work", bufs=2))
        ps_setup = ctx.enter_context(tc.tile_pool(name="ps_setup", bufs=1, space="PSUM"))
        ps_loop = ctx.enter_context(tc.tile_pool(name="ps_loop", bufs=1, space="PSUM"))

        # ---- load constants / weights ----
        tok_sb = const.tile([128, NG * L], i32)
        nc.gpsimd.dma_start(out=tok_sb[:], in_=tok_d)
        consts = const.tile([128, CW], f32)
        nc.sync.dma_start(out=r_(consts[:]), in_=consts_d)
        keysT = consts[0:D, 0:M]
        UwT = consts[0:D, 20:120]
        WwT = consts[0:D, 120:220]
        VwT = consts[0:D, 220:320]
        onesD = consts[0:D, 320:321]
        ones1 = consts[0:1, 321:421]
        ident = consts[0:128, 421:549]

        # ---- Vk = Vw @ keys^T (early; only needs weights) ----
        ps_vk = ps_setup.tile([D, M], f32, tag="psws", name="ps_vk")
        mm(ps_vk[:], VwT, keysT)
        Vk = const.tile([D, M], f32)
        nc.scalar.copy(out=Vk[:], in_=ps_vk[:])
        # bf16 ones row for the cheap 1-cycle/col broadcast matmuls
        bf16 = mybir.dt.bfloat16
        ones1b = const.tile([1, 100], bf16)
        nc.scalar.copy(out=ones1b[:], in_=ones1)

        # ---- gather machinery ----
        # group g covers steps 16g..16g+15 (128 sentences, row p = 8*tg + b);
        # 32 word-gathers accumulate emb-sum and kg into 4 partial tiles.
        parts = [[const.tile([128, EW], f32, name=f"part{g}_{q}",
                             tag=f"part{g % 3}_{q}")
                  for q in range(NPART)] for g in range(NG)]
        sentsS = [const.tile([128, EW], f32, name=f"sentsS{g}", tag=f"sS{g % 2}")
                  for g in range(NG)]
        sentsD = [const.tile([D, 128], f32, name=f"sentsD{g}", tag=f"sD{g % 2}")
                  for g in range(NG)]
        Ws = [const.tile([D, 128], f32, name=f"ws{g}", tag=f"ws{g % 2}")
              for g in range(NG)]
        kgrow = [const.tile([1, GSTEP * R], f32, name=f"kgrow{g}",
                            tag=f"kg{g % 2}") for g in range(NG)]
        ekg = [const.tile([1, GSTEP * R], f32, name=f"ekg{g}",
                          tag=f"ekg{g % 2}") for g in range(NG)]
        vw = [const.tile([D, GSTEP * R], f32, name=f"vw{g}", tag=f"vw{g % 2}")
              for g in range(NG)]

        def emit_gather(g, l):
            q = l % NPART
            nc.gpsimd.indirect_dma_start(
                out=parts[g][q][:],
                out_offset=None,
                in_=emb_d,
                in_offset=bass.IndirectOffsetOnAxis(
                    ap=tok_sb[:, g * L + l:g * L + l + 1], axis=0),
                compute_op=(OP.bypass if l < NPART else OP.add),
            )

        def emit_postproc(g):
            # combine partials -> sentence-major [sents | kg]
            e01 = work.tile([128, EW], f32, tag="comb", name=f"e01_{g}")
            nc.vector.tensor_tensor(out=e01[:], in0=parts[g][0][:],
                                    in1=parts[g][1][:], op=OP.add)
            e23 = work.tile([128, EW], f32, tag="comb", name=f"e23_{g}")
            nc.vector.tensor_tensor(out=e23[:], in0=parts[g][2][:],
                                    in1=parts[g][3][:], op=OP.add)
            nc.vector.tensor_tensor(out=sentsS[g][:], in0=e01[:], in1=e23[:],
                                    op=OP.add)
            # sents -> D-major via PE transpose
            pstr = ps_setup.tile([D, 128], f32, tag="pstr", name=f"pstr{g}")
            nc.tensor.transpose(out=pstr[:], in_=sentsS[g][:, 0:D],
                                identity=ident)
            nc.scalar.copy(out=r_(sentsD[g][:]), in_=pstr[:])
            # Ws = Ww @ sents
            psws = ps_setup.tile([D, 128], f32, tag="psws", name=f"psws{g}")
            mm(psws[:], WwT, sentsD[g][:])
            nc.scalar.copy(out=Ws[g][:], in_=psws[:])
            # kg rows: [128 sent, 20] -> [1, 16*160] via SBUF->SBUF DMA
            nc.sync.dma_start(out=kgrow[g][:], in_=sentsS[g][:, D:D + M])

        def emit_chunk(g, c):
            # ekg chunk: exp(-kg) for steps 4c..4c+3 of group g
            sl = slice(4 * c * R, (4 * c + 4) * R)
            nc.scalar.activation(out=ekg[g][:, sl], in_=kgrow[g][:, sl],
                                 func=AF.Exp, scale=-1.0)
            # vw chunk: Vk + Ws broadcast, [D, 4*160] view [D, 4, 8, 20]
            vsl = vw[g][:, 4 * c * R:(4 * c + 4) * R]
            out = bass.AP(vsl.tensor, vsl.offset,
                          [list(vsl.ap[0]), [R, 4], [M, BL], [1, M]])
            ws_sl = Ws[g][:, 32 * c:32 * c + 32]
            ws3 = bass.AP(ws_sl.tensor, ws_sl.offset,
                          [list(ws_sl.ap[0]), [8, 4], [1, 8]])
            nc.vector.tensor_tensor(out=out, in0=bcast_mid2(Vk[:], 4, BL),
                                    in1=bcast_last3(ws3, M), op=OP.add)

        # ---- lead-in: group 0 gathers + postproc + chunks; group 1 streams
        # during window 0 ----
        for l in range(L):
            emit_gather(0, l)
        emit_postproc(0)
        for c in range(NPART):
            emit_chunk(0, c)
        gcur = [L]                          # global gather cursor (group 0 done)

        # ---- initial state ----
        U = work.tile([D, R], f32, tag="U")
        nc.vector.tensor_copy(out=r_(U[:].rearrange("d (b m) -> d b m", m=M)),
                              in_=bcast_mid(keysT, BL))
        rho = None
        vkwsn = None

        # ---- recurrence ----
        for t in range(n_steps):
            g, k = t // GSTEP, t % GSTEP
            rs = (t % RESCALE == 0)           # U exactly normalized (rho=n=1)
            rescale_now = ((t + 1) % RESCALE == 0)

            # stream remaining gathers continuously (3 per step, global
            # cursor); postproc/chunks pipelined one window ahead
            for _ in range(3):
                if gcur[0] < NG * L:
                    emit_gather(gcur[0] // L, gcur[0] % L)
                    gcur[0] += 1
            gn = g + 1
            if gn < NG:
                if k == 13:
                    emit_postproc(gn)
                elif k == 14:
                    emit_chunk(gn, 0)
                elif k == 15:
                    emit_chunk(gn, 1)
            if g >= 1:
                if k == 0:
                    emit_chunk(g, 2)
                elif k == 1:
                    emit_chunk(g, 3)

            # cand (n-scaled): psA = Uw@U; candf = psA + vw*n
            psA = ps_loop.tile([D, R], f32, tag="psA")
            mm(psA[:], UwT, U[:])

            # gate reduce: psmg = 1^T (U . s_t)
            mgt = work.tile([D, BL, M], f32, tag="mgt")
            nc.vector.tensor_tensor(
                out=r_(mgt[:]),
                in0=U[:].rearrange("d (b m) -> d b m", m=M),
                in1=bcast_last(sentsD[g][:, BL * k:BL * (k + 1)], M),
                op=OP.mult)
            psmg = ps_loop.tile([1, R], f32, tag="psmg")
            mm(psmg[:], onesD, mgt[:].rearrange("d b m -> d (b m)"))

            if rs:
                l_ap = psmg[:]
            else:
                l_sb = work.tile([1, R], f32, tag="l")
                nc.vector.tensor_tensor(out=l_sb[:], in0=psmg[:], in1=rho[:],
                                        op=OP.mult)
                l_ap = l_sb[:]
            e1 = work.tile([1, R], f32, tag="e1")
            nc.scalar.activation(out=e1[:], in_=l_ap, func=AF.Exp, scale=-1.0)
            e_sb = work.tile([1, R], bf16, tag="e")
            nc.vector.tensor_tensor(out=e_sb[:], in0=e1[:],
                                    in1=ekg[g][:, k * R:(k + 1) * R], op=OP.mult)

            # candf = psA + vw_t * n  (vkwsn from prev tail; raw vw when n=1)
            candf = work.tile([D, R], f32, tag="candf")
            if rs:
                nc.vector.tensor_tensor(out=candf[:], in0=psA[:],
                                        in1=vw[g][:, k * R:(k + 1) * R],
                                        op=OP.add)
            else:
                nc.vector.tensor_tensor(out=candf[:], in0=psA[:],
                                        in1=vkwsn, op=OP.add)

            # U' = (1 + e) . U + candf
            psbce = ps_loop.tile([D, R], f32, tag="psbce")
            nc.tensor.matmul(out=psbce[:], lhsT=ones1b[:], rhs=e_sb[:],
                             start=True, stop=True)
            V_sb = work.tile([D, R], f32, tag="V")
            nc.vector.scalar_tensor_tensor(out=V_sb[:], in0=psbce[:],
                                           scalar=1.0, in1=U[:],
                                           op0=OP.add, op1=OP.mult)
            U2 = work.tile([D, R], f32, tag="U")
            nc.vector.tensor_tensor(out=r_(U2[:]), in0=V_sb[:], in1=candf[:],
                                    op=OP.add)
            U = U2

            # norms: rho' = exp(-0.5 ln ss), n' = ss * rho'
            sq = work.tile([D, R], f32, tag="sq")
            nc.vector.tensor_tensor(out=r_(sq[:]), in0=U[:], in1=U[:],
                                    op=OP.mult)
            psss = ps_loop.tile([1, R], f32, tag="psss")
            mm(psss[:], onesD, sq[:])
            lnss = work.tile([1, R], f32, tag="lnss")
            nc.scalar.activation(out=lnss[:], in_=psss[:], func=AF.Ln)
            rho2 = work.tile([1, R], f32, tag="rho")
            nc.scalar.activation(out=r_(rho2[:]), in_=lnss[:], func=AF.Exp,
                                 scale=-0.5)
            rho = rho2

            if rescale_now:
                # exact renormalization: U *= bc(rho); afterwards rho = n = 1
                psbcr = ps_loop.tile([D, R], f32, tag="psbcn", name=f"psbcr{t}")
                mm(psbcr[:], ones1, rho[:])
                U3 = work.tile([D, R], f32, tag="U")
                nc.vector.tensor_tensor(out=r_(U3[:]), in0=psbcr[:], in1=U[:],
                                        op=OP.mult)
                U = U3
            elif t < n_steps - 1:
                # n' for next step's vkwsn
                n_sb = work.tile([1, R], bf16, tag="n")
                nc.vector.tensor_tensor(out=n_sb[:], in0=psss[:], in1=rho[:],
                                        op=OP.mult)
                psbcn = ps_loop.tile([D, R], f32, tag="psbcn")
                nc.tensor.matmul(out=psbcn[:], lhsT=ones1b[:], rhs=n_sb[:],
                                 start=True, stop=True)
                tn = t + 1
                gg, kk = tn // GSTEP, tn % GSTEP
                vkwsn2 = work.tile([D, R], f32, tag="vkwsn")
                nc.vector.tensor_tensor(out=vkwsn2[:], in0=psbcn[:],
                                        in1=vw[gg][:, kk * R:(kk + 1) * R],
                                        op=OP.mult)
                vkwsn = vkwsn2[:]

        # ---- output ----
        if n_steps % RESCALE == 0:
            nc.sync.dma_start(out=out_d, in_=U[:])
        else:
            psbcr = ps_loop.tile([D, R], f32, tag="psbcn")
            mm(psbcr[:], ones1, rho[:])
            memT = work.tile([D, R], f32, tag="memT")
            nc.vector.tensor_tensor(out=memT[:], in0=psbcr[:], in1=U[:],
                                    op=OP.mult)
            nc.sync.dma_start(out=out_d, in_=memT[:])

    _strip_redundant_self_waits(nc)
    return nc


def _np_fallback(tokens, emb, keys, mult, Uw, Vw, Ww, prelu_a):
    tokens = np.asarray(tokens)
    emb = np.asarray(emb, np.float32)
    keys = np.asarray(keys, np.float32)
    mult = np.asarray(mult, np.float32)
    Uw, Vw, Ww = (np.asarray(x, np.float32) for x in (Uw, Vw, Ww))
    a = float(np.asarray(prelu_a).reshape(-1)[0])
    x = emb[tokens] * mult
    sents = x.sum(axis=2, dtype=np.float32)
    mem = np.broadcast_to(keys, (tokens.shape[0], M, D)).astype(np.float32).copy()
    Vk = keys @ Vw.T
    for t in range(tokens.shape[1]):
        s = sents[:, t, :]
        logits = np.einsum('bd,bmd->bm', s, mem) + s @ keys.T
        gate = 1.0 / (1.0 + np.exp(-logits))
        pre = mem @ Uw.T + Vk + (s @ Ww.T)[:, None, :]
        cand = np.where(pre >= 0, pre, a * pre)
        mask = np.where(logits == 0.0, 0.0, 1.0)
        mem = mem + cand * (gate * mask)[:, :, None]
        mem = mem / np.linalg.norm(mem, axis=2, keepdims=True)
    return mem.astype(np.float32)


def _stage_inputs(tokens, emb, keys, mult, Uw, Vw, Ww, prelu_a):
    """Host-side sharding/layout prep. Returns (in_maps, flags)."""
    tokens = np.asarray(tokens)
    emb = np.asarray(emb, dtype=np.float32)
    keys = np.asarray(keys, dtype=np.float32)
    mult = np.asarray(mult, dtype=np.float32)
    a = float(np.asarray(prelu_a).reshape(-1)[0])
    a_is_one = (a == 1.0)
    mult_is_ones = bool(np.all(mult == 1.0))

    embaug = np.zeros((V, EW), np.float32)
    embaug[:, 0:D] = emb
    embaug[:, D:D + M] = emb @ keys.T
    embaug = np.ascontiguousarray(embaug)

    CW = 549
    consts = np.zeros((128, CW), np.float32)
    consts[0:D, 0:M] = keys.T
    consts[0:D, 20:120] = np.asarray(Uw, np.float32).T        # lhsT for Uw@mem
    consts[0:D, 120:220] = np.asarray(Ww, np.float32).T
    consts[0:D, 220:320] = np.asarray(Vw, np.float32).T
    consts[0:D, 320:321] = 1.0                                # onesD
    consts[0:1, 321:421] = 1.0                                # ones1
    consts[0:128, 421:549] = np.eye(128, dtype=np.float32)    # ident (transpose)

    in_maps = []
    for c in range(NCORES):
        tc_ = tokens[c * BL:(c + 1) * BL]                     # [8, S, L]
        # tok_staged[p, 32g + l] = tokens[b=p%8, t=16g+p//8, l]
        arr = tc_.reshape(BL, NG, GSTEP, L).transpose(1, 2, 0, 3)  # [g,tg,b,l]
        arr = arr.reshape(NG, 128, L).transpose(1, 0, 2).reshape(128, NG * L)
        in_maps.append({"tok": np.ascontiguousarray(arr, np.int32),
                        "embaug": embaug, "consts": consts})
    return in_maps, a_is_one, mult_is_ones, a


def kernel(tokens, emb, keys, mult, Uw, Vw, Ww, prelu_a, _trace=False):
    from concourse.bass_utils import run_bass_kernel_spmd

    in_maps, a_is_one, mult_is_ones, a = _stage_inputs(
        tokens, emb, keys, mult, Uw, Vw, Ww, prelu_a)
    if not (a_is_one and mult_is_ones):
        return _np_fallback(tokens, emb, keys, mult, Uw, Vw, Ww, prelu_a)

    key = (a_is_one, mult_is_ones)
    if key not in _prog_cache:
        _prog_cache[key] = _build_program(a_is_one, mult_is_ones, a)
    nc = _prog_cache[key]

    res = run_bass_kernel_spmd(nc, in_maps, list(range(NCORES)), trace=_trace)
    out = np.empty((B, M, D), dtype=np.float32)
    for c in range(NCORES):
        memT = res.results[c]["memT"]                          # [D, R]
        out[c * BL:(c + 1) * BL] = memT.reshape(D, BL, M).transpose(1, 2, 0)
    kernel._last_results = res
    return out


# revision 11
# speedup vs baseline: 1.1783x; 1.1619x over previous
"""EntNet Trainium2 kernel (8-core data-parallel over batch).

Reference computation (shapes: B=64, S=128, L=32, D=100, M=20, V=50000):
  sents = (emb[tokens] * mult).sum(axis=2)            # [B,S,D]
  mem0 = broadcast(keys)                              # [B,M,D]
  per step t: gate = sigmoid(s.mem + s.keys); cand = prelu(mem@Uw.T + keys@Vw.T + s@Ww.T)
              mem = normalize(mem + cand*gate, axis=D)

Kernel strategy per core (8 batches/core, R = 8*20 = 160 (b,m) rows):
  - Embedding gather via gpsimd indirect DMA (128 tokens = 4 sentences per
    instruction), word-summed by one tiny f32r matmul per gather straight
    into a per-group D-major psum [100,128] (Tile's scheduler models bypass
    gathers accurately, so it interleaves the stream without queue stalls;
    accumulating gathers run 60% over model and head-of-line block queues).
  - Key-gate logits kg = s.keys precomputed per group from sents via
    keys-broadcast DVE products reduced by [1,320] f32r matmuls.
  - Recurrence kept in scale-free form: with U unnormalized and
    rho = 1/||U|| per row, the update
        mem' = normalize(mem + cand * sigmoid(l))
    is exactly
        U' = (1 + exp(-l)) . U + Uw@U + (Vk + Ws_t) * n        (n = ||U||)
    Gate split: exp(-l) = exp(-rho * 1^T(U.s)) * exp(-kg), with exp(-kg)
    precomputed per 16-step group (kg rows extracted by a SBUF->SBUF DMA
    transpose-gather), so no key-term matmul and no n-scaled keys on the
    critical path.
  - All matmuls run as float32r (single-pass PE mode) instead of float32
    (which costs 2 half-rate passes + 2 LDWEIGHTS each).
"""

import numpy as np

B, S, L, D, M, V = 64, 128, 32, 100, 20, 50000
NCORES = 8
BL = B // NCORES            # 8 batches per core
NS = BL * S                 # 1024 sentences per core
R = BL * M                  # 160 (b, m) rows per core
NG = 8                      # gather groups (16 steps each)
GSTEP = S // NG             # 16 steps per group
GPC = 32                    # gathers per group (4 sentences each)
RESCALE = 8                 # renormalize U every RESCALE steps (f32 range)

_prog_cache = {}

_ENGINE_SEM = {"PE": "PE_", "DVE": "DVE_", "Activation": "Activation_",
               "Pool": "Pool_", "SP": "SP_"}


def _strip_redundant_self_waits(nc):
    """Legalize sync waits: walrus rejects >1 sync wait on most instruction
    structs. For any instruction carrying several, hoist all but one onto
    preceding single-wait NoOps on the same engine queue (in-order dispatch
    keeps semantics). The instruction keeps its OWN-engine wait if it has one
    (that wait guards an engine-pipelining RAW hazard and must gate execution,
    not just dispatch).
    """
    import concourse.mybir as mybir
    for fn in nc.m.functions:
        for blk in fn.blocks:
            i = 0
            while i < len(blk.instructions):
                inst = blk.instructions[i]
                si = inst.sync_info() if callable(inst.sync_info) else inst.sync_info
                if (si is not None and si.on_wait and len(si.on_wait) > 1
                        and inst.engine is not None):
                    waits = list(si.on_wait)
                    pref = _ENGINE_SEM.get(inst.engine.name)
                    keep_idx = None
                    for j, w in enumerate(waits):
                        if pref and w.ant_name.startswith(pref):
                            keep_idx = j
                            break
                    kept = [waits.pop(keep_idx)] if keep_idx is not None else []
                    noops = []
                    for w in waits:
                        nop = mybir.InstNoOp(
                            name=nc.get_next_instruction_name(), ins=[], outs=[])
                        nop.engine = inst.engine
                        nop.sync_info = mybir.SyncInfo(on_wait=[w], on_update=[])
                        nc.register_instruction(nop, overwrite=True)
                        noops.append(nop)
                    inst.sync_info = mybir.SyncInfo(
                        on_wait=kept, on_update=list(si.on_update))
                    blk.instructions[i:i] = noops
                    i += len(noops)
                i += 1


def _build_program(a_is_one: bool, mult_is_ones: bool, alpha: float,
                   n_steps: int = S, dump: bool = False):
    import concourse.bass as bass
    import concourse.tile as tile
    from concourse import mybir
    from contextlib import ExitStack

    assert mult_is_ones and a_is_one, "fast path only (host fallback otherwise)"

    f32 = mybir.dt.float32
    f32r = mybir.dt.float32r
    i32 = mybir.dt.int32
    AF = mybir.ActivationFunctionType
    OP = mybir.AluOpType

    nc = bass.Bass(trn_type="TRN2")

    # ---- DRAM I/O ----
    CW = 549
    tok_d = nc.dram_tensor("tok", [128, NG * L], i32, kind="ExternalInput").ap()
    emb_d = nc.dram_tensor("emb", [V, D], f32r, kind="ExternalInput").ap()
    consts_d = nc.dram_tensor("consts", [128, CW], f32r, kind="ExternalInput").ap()
    out_d = nc.dram_tensor("memT", [D, R], f32, kind="ExternalOutput").ap()

    def r_(ap):
        # f32r = fp32 bits fed to the PE in single-pass (replicated) mode.
        # walrus's BIR verifier requires producers of f32r-matmul operands to
        # declare f32r-rounded output, so producer OUT aps are bitcast too.
        return ap.bitcast(f32r)

    def mm(out, lhsT, rhs, start=True, stop=True):
        nc.tensor.matmul(out=out, lhsT=r_(lhsT), rhs=r_(rhs),
                         start=start, stop=stop)

    def bcast_mid(ap_2d, n_mid):
        # [P, k] -> [P, n_mid, k] with stride-0 middle dim
        return bass.AP(ap_2d.tensor, ap_2d.offset,
                       [list(ap_2d.ap[0]), [0, n_mid], list(ap_2d.ap[1])])

    def bcast_last(ap_2d, n_last):
        # [P, k] -> [P, k, n_last] with stride-0 last dim
        return bass.AP(ap_2d.tensor, ap_2d.offset,
                       [list(ap_2d.ap[0]), list(ap_2d.ap[1]), [0, n_last]])

    def bcast_mid2(ap_2d, n1, n2):
        # [P, k] -> [P, n1, n2, k] with stride-0 middle dims
        return bass.AP(ap_2d.tensor, ap_2d.offset,
                       [list(ap_2d.ap[0]), [0, n1], [0, n2], list(ap_2d.ap[1])])

    def bcast_last3(ap_3d, n_last):
        # [P, a, b] -> [P, a, b, n_last] with stride-0 last dim
        return bass.AP(ap_3d.tensor, ap_3d.offset,
                       [list(ap_3d.ap[0]), list(ap_3d.ap[1]),
                        list(ap_3d.ap[2]), [0, n_last]])

    with tile.TileContext(nc) as tc, ExitStack() as ctx:
        const = ctx.enter_context(tc.tile_pool(name="const", bufs=1))
        work = ctx.enter_context(tc.tile_pool(name="work", bufs=2))
        ps_setup = ctx.enter_context(tc.tile_pool(name="ps_setup", bufs=1, space="PSUM"))
        ps_loop = ctx.enter_context(tc.tile_pool(name="ps_loop", bufs=1, space="PSUM"))

        # ---- load constants / weights ----
        tok_sb = const.tile([128, NG * L], i32)
        nc.gpsimd.dma_start(out=tok_sb[:], in_=tok_d)
        consts = const.tile([128, CW], f32)
        nc.sync.dma_start(out=r_(consts[:]), in_=consts_d)
        keysT = consts[0:D, 0:M]
        UwT = consts[0:D, 20:120]
        WwT = consts[0:D, 120:220]
        VwT = consts[0:D, 220:320]
        onesD = consts[0:D, 320:321]
        ones1 = consts[0:1, 321:421]
        blk = consts[0:128, 421:425]

        # ---- Vk = Vw @ keys^T (early; only needs weights) ----
        ps_vk = ps_setup.tile([D, M], f32, tag="psws", name="ps_vk")
        mm(ps_vk[:], VwT, keysT)
        Vk = const.tile([D, M], f32)
        nc.scalar.copy(out=Vk[:], in_=ps_vk[:])
        # bf16 ones row for the cheap 1-cycle/col broadcast matmuls
        bf16 = mybir.dt.bfloat16
        ones1b = const.tile([1, 100], bf16)
        nc.scalar.copy(out=ones1b[:], in_=ones1)

        # ---- gather machinery ----
        # group g covers steps 16g..16g+15 = sentences 128g..128g+127
        # (t-major: sentence j = 8t+b). gather c covers sentences 4c..4c+3:
        # 128 tokens, one per partition; word-sum via f32r matmul with the
        # block-ones matrix into the group psum (D-major, cols = sentence).
        gpool = ctx.enter_context(tc.tile_pool(name="gath", bufs=8))
        sentsD = [const.tile([D, 128], f32, name=f"sentsD{g}", tag=f"sD{g % 2}")
                  for g in range(NG)]
        Ws = [const.tile([D, 128], f32, name=f"ws{g}", tag=f"ws{g % 2}")
              for g in range(NG)]
        ekg = [const.tile([1, GSTEP * R], f32, name=f"ekg{g}",
                          tag=f"ekg{g % 2}") for g in range(NG)]
        vw = [const.tile([D, GSTEP * R], f32, name=f"vw{g}", tag=f"vw{g % 2}")
              for g in range(NG)]
        ps_grp = [ps_setup.tile([D, 128], f32, tag="psgrp", name=f"psgrp{g}")
                  for g in range(NG)]

        def emit_gather(c):
            g = c // GPC
            j = c % GPC
            gt = gpool.tile([128, D], f32, tag="g", name=f"g{c}")
            nc.gpsimd.indirect_dma_start(
                out=r_(gt[:]),
                out_offset=None,
                in_=emb_d,
                in_offset=bass.IndirectOffsetOnAxis(
                    ap=tok_sb[:, c:c + 1], axis=0),
            )
            mm(ps_grp[g][:, 4 * j:4 * j + 4], gt[:], blk)

        def emit_finalize(g):
            nc.scalar.copy(out=r_(sentsD[g][:]), in_=ps_grp[g][:])
            psws = ps_setup.tile([D, 128], f32, tag="psws", name=f"psws{g}")
            mm(psws[:], WwT, sentsD[g][:])
            nc.scalar.copy(out=Ws[g][:], in_=psws[:])

        def emit_chunk(g, c):
            # ksraw chunk: keys.s products for steps 4c..4c+3, reduced to
            # kg rows by two [1,320] f32r matmuls, then ekg = exp(-kg)
            ksr = work.tile([D, 4 * R], f32, tag="ksr", name=f"ksr{g}_{c}")
            kv = bass.AP(ksr.tensor, ksr.offset,
                         [list(ksr[:].ap[0]), [R, 4], [M, BL], [1, M]])
            s_sl = sentsD[g][:, 32 * c:32 * c + 32]
            s3 = bass.AP(s_sl.tensor, s_sl.offset,
                         [list(s_sl.ap[0]), [8, 4], [1, 8]])
            nc.vector.tensor_tensor(out=kv.bitcast(f32r),
                                    in0=bcast_mid2(keysT, 4, BL),
                                    in1=bcast_last3(s3, M), op=OP.mult)
            ksf = ksr[:]
            for h in range(2):
                pskg = ps_loop.tile([1, 2 * R], f32, tag="pskg",
                                    name=f"pskg{g}_{c}_{h}")
                mm(pskg[:], onesD, ksf[:, 2 * R * h:2 * R * (h + 1)])
                nc.scalar.activation(
                    out=ekg[g][:, 4 * c * R + 2 * R * h:4 * c * R + 2 * R * (h + 1)],
                    in_=pskg[:], func=AF.Exp, scale=-1.0)
            # vw chunk: Vk + Ws broadcast, [D, 4*160] view [D, 4, 8, 20]
            vsl = vw[g][:, 4 * c * R:(4 * c + 4) * R]
            out = bass.AP(vsl.tensor, vsl.offset,
                          [list(vsl.ap[0]), [R, 4], [M, BL], [1, M]])
            ws_sl = Ws[g][:, 32 * c:32 * c + 32]
            ws3 = bass.AP(ws_sl.tensor, ws_sl.offset,
                          [list(ws_sl.ap[0]), [8, 4], [1, 8]])
            nc.vector.tensor_tensor(out=out, in0=bcast_mid2(Vk[:], 4, BL),
                                    in1=bcast_last3(ws3, M), op=OP.add)

        # ---- lead-in: group 0 gathers + postproc + chunks; group 1 streams
        # during window 0 ----
        for c in range(GPC):
            emit_gather(c)
        emit_finalize(0)
        for c in range(4):
            emit_chunk(0, c)
        gcur = [GPC]                        # global gather cursor (group 0 done)

        # ---- initial state ----
        U = work.tile([D, R], f32, tag="U")
        nc.vector.tensor_copy(out=r_(U[:].rearrange("d (b m) -> d b m", m=M)),
                              in_=bcast_mid(keysT, BL))
        rho = None
        vkwsn = None

        # ---- recurrence ----
        for t in range(n_steps):
            g, k = t // GSTEP, t % GSTEP
            rs = (t % RESCALE == 0)           # U exactly normalized (rho=n=1)
            rescale_now = ((t + 1) % RESCALE == 0)

            # stream remaining gathers continuously (3 per step, global
            # cursor); finalize/chunks pipelined one window ahead
            for _ in range(3):
                if gcur[0] < NG * GPC:
                    emit_gather(gcur[0])
                    gcur[0] += 1
            gn = g + 1
            if gn < NG:
                if k == 11:
                    emit_finalize(gn)
                elif k >= 12:
                    emit_chunk(gn, k - 12)

            # cand (n-scaled): psA = Uw@U; candf = psA + vw*n
            psA = ps_loop.tile([D, R], f32, tag="psA")
            mm(psA[:], UwT, U[:])

            # gate reduce: psmg = 1^T (U . s_t)
            mgt = work.tile([D, BL, M], f32, tag="mgt")
            nc.vector.tensor_tensor(
                out=r_(mgt[:]),
                in0=U[:].rearrange("d (b m) -> d b m", m=M),
                in1=bcast_last(sentsD[g][:, BL * k:BL * (k + 1)], M),
                op=OP.mult)
            psmg = ps_loop.tile([1, R], f32, tag="psmg")
            mm(psmg[:], onesD, mgt[:].rearrange("d b m -> d (b m)"))

            if rs:
                l_ap = psmg[:]
            else:
                l_sb = work.tile([1, R], f32, tag="l")
                nc.vector.tensor_tensor(out=l_sb[:], in0=psmg[:], in1=rho[:],
                                        op=OP.mult)
                l_ap = l_sb[:]
            e1 = work.tile([1, R], f32, tag="e1")
            nc.scalar.activation(out=e1[:], in_=l_ap, func=AF.Exp, scale=-1.0)
            e_sb = work.tile([1, R], bf16, tag="e")
            nc.vector.tensor_tensor(out=e_sb[:], in0=e1[:],
                                    in1=ekg[g][:, k * R:(k + 1) * R], op=OP.mult)

            # candf = psA + vw_t * n  (vkwsn from prev tail; raw vw when n=1)
            candf = work.tile([D, R], f32, tag="candf")
            if rs:
                nc.vector.tensor_tensor(out=candf[:], in0=psA[:],
                                        in1=vw[g][:, k * R:(k + 1) * R],
                                        op=OP.add)
            else:
                nc.vector.tensor_tensor(out=candf[:], in0=psA[:],
                                        in1=vkwsn, op=OP.add)

            # U' = (1 + e) . U + candf
            psbce = ps_loop.tile([D, R], f32, tag="psbce")
            nc.tensor.matmul(out=psbce[:], lhsT=ones1b[:], rhs=e_sb[:],
                             start=True, stop=True)
            V_sb = work.tile([D, R], f32, tag="V")
            nc.vector.scalar_tensor_tensor(out=V_sb[:], in0=psbce[:],
                                           scalar=1.0, in1=U[:],
                                           op0=OP.add, op1=OP.mult)
            U2 = work.tile([D, R], f32, tag="U")
            nc.vector.tensor_tensor(out=r_(U2[:]), in0=V_sb[:], in1=candf[:],
                                    op=OP.add)
            U = U2

            # norms: rho' = exp(-0.5 ln ss), n' = ss * rho'
            sq = work.tile([D, R], f32, tag="sq")
            nc.vector.tensor_tensor(out=r_(sq[:]), in0=U[:], in1=U[:],
                                    op=OP.mult)
            psss = ps_loop.tile([1, R], f32, tag="psss")
            mm(psss[:], onesD, sq[:])
            lnss = work.tile([1, R], f32, tag="lnss")
            nc.scalar.activation(out=lnss[:], in_=psss[:], func=AF.Ln)
            rho2 = work.tile([1, R], f32, tag="rho")
            nc.scalar.activation(out=r_(rho2[:]), in_=lnss[:], func=AF.Exp,
                                 scale=-0.5)
            rho = rho2

            if rescale_now:
                # exact renormalization: U *= bc(rho); afterwards rho = n = 1
                psbcr = ps_loop.tile([D, R], f32, tag="psbcn", name=f"psbcr{t}")
                mm(psbcr[:], ones1, rho[:])
                U3 = work.tile([D, R], f32, tag="U")
                nc.vector.tensor_tensor(out=r_(U3[:]), in0=psbcr[:], in1=U[:],
                                        op=OP.mult)
                U = U3
            elif t < n_steps - 1:
                # n' for next step's vkwsn
                n_sb = work.tile([1, R], bf16, tag="n")
                nc.vector.tensor_tensor(out=n_sb[:], in0=psss[:], in1=rho[:],
                                        op=OP.mult)
                psbcn = ps_loop.tile([D, R], f32, tag="psbcn")
                nc.tensor.matmul(out=psbcn[:], lhsT=ones1b[:], rhs=n_sb[:],
                                 start=True, stop=True)
                tn = t + 1
                gg, kk = tn // GSTEP, tn % GSTEP
                vkwsn2 = work.tile([D, R], f32, tag="vkwsn")
                nc.vector.tensor_tensor(out=vkwsn2[:], in0=psbcn[:],
                                        in1=vw[gg][:, kk * R:(kk + 1) * R],
                                        op=OP.mult)
                vkwsn = vkwsn2[:]

        # ---- output ----
        if n_steps % RESCALE == 0:
            nc.sync.dma_start(out=out_d, in_=U[:])
        else:
            psbcr = ps_loop.tile([D, R], f32, tag="psbcn")
            mm(psbcr[:], ones1, rho[:])
            memT = work.tile([D, R], f32, tag="memT")
            nc.vector.tensor_tensor(out=memT[:], in0=psbcr[:], in1=U[:],
                                    op=OP.mult)
            nc.sync.dma_start(out=out_d, in_=memT[:])

    _strip_redundant_self_waits(nc)
    return nc


def _np_fallback(tokens, emb, keys, mult, Uw, Vw, Ww, prelu_a):
    tokens = np.asarray(tokens)
    emb = np.asarray(emb, np.float32)
    keys = np.asarray(keys, np.float32)
    mult = np.asarray(mult, np.float32)
    Uw, Vw, Ww = (np.asarray(x, np.float32) for x in (Uw, Vw, Ww))
    a = float(np.asarray(prelu_a).reshape(-1)[0])
    x = emb[tokens] * mult
    sents = x.sum(axis=2, dtype=np.float32)
    mem = np.broadcast_to(keys, (tokens.shape[0], M, D)).astype(np.float32).copy()
    Vk = keys @ Vw.T
    for t in range(tokens.shape[1]):
        s = sents[:, t, :]
        logits = np.einsum('bd,bmd->bm', s, mem) + s @ keys.T
        gate = 1.0 / (1.0 + np.exp(-logits))
        pre = mem @ Uw.T + Vk + (s @ Ww.T)[:, None, :]
        cand = np.where(pre >= 0, pre, a * pre)
        mask = np.where(logits == 0.0, 0.0, 1.0)
        mem = mem + cand * (gate * mask)[:, :, None]
        mem = mem / np.linalg.norm(mem, axis=2, keepdims=True)
    return mem.astype(np.float32)


def _stage_inputs(tokens, emb, keys, mult, Uw, Vw, Ww, prelu_a):
    """Host-side sharding/layout prep. Returns (in_maps, flags)."""
    tokens = np.asarray(tokens)
    emb = np.asarray(emb, dtype=np.float32)
    keys = np.asarray(keys, dtype=np.float32)
    mult = np.asarray(mult, dtype=np.float32)
    a = float(np.asarray(prelu_a).reshape(-1)[0])
    a_is_one = (a == 1.0)
    mult_is_ones = bool(np.all(mult == 1.0))

    emb = np.ascontiguousarray(emb)

    CW = 549
    consts = np.zeros((128, CW), np.float32)
    consts[0:D, 0:M] = keys.T
    consts[0:D, 20:120] = np.asarray(Uw, np.float32).T        # lhsT for Uw@mem
    consts[0:D, 120:220] = np.asarray(Ww, np.float32).T
    consts[0:D, 220:320] = np.asarray(Vw, np.float32).T
    consts[0:D, 320:321] = 1.0                                # onesD
    consts[0:1, 321:421] = 1.0                                # ones1
    consts[0:128, 421:425] = np.kron(np.eye(4, dtype=np.float32),
                                     np.ones((L, 1), np.float32))  # blk

    in_maps = []
    for c in range(NCORES):
        tc_ = tokens[c * BL:(c + 1) * BL]                     # [8, S, L]
        # sentence-major rows with t-major sentence order: row j = 8t+b
        tokflat = np.ascontiguousarray(tc_.transpose(1, 0, 2)).reshape(NS, L)
        # tok_staged[p, c] = token of sentence 4c + p//32, word p%32
        tok_staged = np.ascontiguousarray(
            tokflat.reshape(2 * S, 4, L).transpose(1, 2, 0)).reshape(128, 2 * S)
        in_maps.append({"tok": np.ascontiguousarray(tok_staged, np.int32),
                        "emb": emb, "consts": consts})
    return in_maps, a_is_one, mult_is_ones, a


def kernel(tokens, emb, keys, mult, Uw, Vw, Ww, prelu_a, _trace=False):
    from concourse.bass_utils import run_bass_kernel_spmd

    in_maps, a_is_one, mult_is_ones, a = _stage_inputs(
        tokens, emb, keys, mult, Uw, Vw, Ww, prelu_a)
    if not (a_is_one and mult_is_ones):
        return _np_fallback(tokens, emb, keys, mult, Uw, Vw, Ww, prelu_a)

    key = (a_is_one, mult_is_ones)
    if key not in _prog_cache:
        _prog_cache[key] = _build_program(a_is_one, mult_is_ones, a)
    nc = _prog_cache[key]

    res = run_bass_kernel_spmd(nc, in_maps, list(range(NCORES)), trace=_trace)
    out = np.empty((B, M, D), dtype=np.float32)
    for c in range(NCORES):
        memT = res.results[c]["memT"]                          # [D, R]
        out[c * BL:(c + 1) * BL] = memT.reshape(D, BL, M).transpose(1, 2, 0)
    kernel._last_results = res
    return out


# revision 12
# speedup vs baseline: 1.1906x; 1.0105x over previous
"""EntNet Trainium2 kernel (8-core data-parallel over batch).

Reference computation (shapes: B=64, S=128, L=32, D=100, M=20, V=50000):
  sents = (emb[tokens] * mult).sum(axis=2)            # [B,S,D]
  mem0 = broadcast(keys)                              # [B,M,D]
  per step t: gate = sigmoid(s.mem + s.keys); cand = prelu(mem@Uw.T + keys@Vw.T + s@Ww.T)
              mem = normalize(mem + cand*gate, axis=D)

Kernel strategy per core (8 batches/core, R = 8*20 = 160 (b,m) rows):
  - Embedding gather via gpsimd indirect DMA (128 tokens = 4 sentences per
    instruction), word-summed by one tiny f32r matmul per gather straight
    into a per-group D-major psum [100,128] (Tile's scheduler models bypass
    gathers accurately, so it interleaves the stream without queue stalls;
    accumulating gathers run 60% over model and head-of-line block queues).
  - Key-gate logits kg = s.keys precomputed per group from sents via
    keys-broadcast DVE products reduced by [1,320] f32r matmuls.
  - Recurrence kept in scale-free form: with U unnormalized and
    rho = 1/||U|| per row, the update
        mem' = normalize(mem + cand * sigmoid(l))
    is exactly
        U' = (1 + exp(-l)) . U + Uw@U + (Vk + Ws_t) * n        (n = ||U||)
    Gate split: exp(-l) = exp(-rho * 1^T(U.s)) * exp(-kg), with exp(-kg)
    precomputed per 16-step group (kg rows extracted by a SBUF->SBUF DMA
    transpose-gather), so no key-term matmul and no n-scaled keys on the
    critical path.
  - All matmuls run as float32r (single-pass PE mode) instead of float32
    (which costs 2 half-rate passes + 2 LDWEIGHTS each).
"""

import numpy as np

B, S, L, D, M, V = 64, 128, 32, 100, 20, 50000
NCORES = 8
BL = B // NCORES            # 8 batches per core
NS = BL * S                 # 1024 sentences per core
R = BL * M                  # 160 (b, m) rows per core
NG = 8                      # gather groups (16 steps each)
GSTEP = S // NG             # 16 steps per group
GPC = 32                    # gathers per group (4 sentences each)
RESCALE = 8                 # renormalize U every RESCALE steps (f32 range)

_prog_cache = {}

_ENGINE_SEM = {"PE": "PE_", "DVE": "DVE_", "Activation": "Activation_",
               "Pool": "Pool_", "SP": "SP_"}


def _strip_redundant_self_waits(nc):
    """Legalize sync waits: walrus rejects >1 sync wait on most instruction
    structs. For any instruction carrying several, hoist all but one onto
    preceding single-wait NoOps on the same engine queue (in-order dispatch
    keeps semantics). The instruction keeps its OWN-engine wait if it has one
    (that wait guards an engine-pipelining RAW hazard and must gate execution,
    not just dispatch).
    """
    import concourse.mybir as mybir
    for fn in nc.m.functions:
        for blk in fn.blocks:
            i = 0
            while i < len(blk.instructions):
                inst = blk.instructions[i]
                si = inst.sync_info() if callable(inst.sync_info) else inst.sync_info
                if (si is not None and si.on_wait and len(si.on_wait) > 1
                        and inst.engine is not None):
                    waits = list(si.on_wait)
                    pref = _ENGINE_SEM.get(inst.engine.name)
                    keep_idx = None
                    for j, w in enumerate(waits):
                        if pref and w.ant_name.startswith(pref):
                            keep_idx = j
                            break
                    kept = [waits.pop(keep_idx)] if keep_idx is not None else []
                    noops = []
                    for w in waits:
                        nop = mybir.InstNoOp(
                            name=nc.get_next_instruction_name(), ins=[], outs=[])
                        nop.engine = inst.engine
                        nop.sync_info = mybir.SyncInfo(on_wait=[w], on_update=[])
                        nc.register_instruction(nop, overwrite=True)
                        noops.append(nop)
                    inst.sync_info = mybir.SyncInfo(
                        on_wait=kept, on_update=list(si.on_update))
                    blk.instructions[i:i] = noops
                    i += len(noops)
                i += 1


def _build_program(a_is_one: bool, mult_is_ones: bool, alpha: float,
                   n_steps: int = S, dump: bool = False):
    import concourse.bass as bass
    import concourse.tile as tile
    from concourse import mybir
    from contextlib import ExitStack

    assert mult_is_ones and a_is_one, "fast path only (host fallback otherwise)"

    f32 = mybir.dt.float32
    f32r = mybir.dt.float32r
    i32 = mybir.dt.int32
    AF = mybir.ActivationFunctionType
    OP = mybir.AluOpType

    nc = bass.Bass(trn_type="TRN2")

    # ---- DRAM I/O ----
    CW = 549
    tok_d = nc.dram_tensor("tok", [128, NG * L], i32, kind="ExternalInput").ap()
    emb_d = nc.dram_tensor("emb", [V, D], f32r, kind="ExternalInput").ap()
    consts_d = nc.dram_tensor("consts", [128, CW], f32r, kind="ExternalInput").ap()
    out_d = nc.dram_tensor("memT", [D, R], f32, kind="ExternalOutput").ap()

    def r_(ap):
        # f32r = fp32 bits fed to the PE in single-pass (replicated) mode.
        # walrus's BIR verifier requires producers of f32r-matmul operands to
        # declare f32r-rounded output, so producer OUT aps are bitcast too.
        return ap.bitcast(f32r)

    def mm(out, lhsT, rhs, start=True, stop=True):
        nc.tensor.matmul(out=out, lhsT=r_(lhsT), rhs=r_(rhs),
                         start=start, stop=stop)

    def bcast_mid(ap_2d, n_mid):
        # [P, k] -> [P, n_mid, k] with stride-0 middle dim
        return bass.AP(ap_2d.tensor, ap_2d.offset,
                       [list(ap_2d.ap[0]), [0, n_mid], list(ap_2d.ap[1])])

    def bcast_last(ap_2d, n_last):
        # [P, k] -> [P, k, n_last] with stride-0 last dim
        return bass.AP(ap_2d.tensor, ap_2d.offset,
                       [list(ap_2d.ap[0]), list(ap_2d.ap[1]), [0, n_last]])

    def bcast_mid2(ap_2d, n1, n2):
        # [P, k] -> [P, n1, n2, k] with stride-0 middle dims
        return bass.AP(ap_2d.tensor, ap_2d.offset,
                       [list(ap_2d.ap[0]), [0, n1], [0, n2], list(ap_2d.ap[1])])

    def bcast_last3(ap_3d, n_last):
        # [P, a, b] -> [P, a, b, n_last] with stride-0 last dim
        return bass.AP(ap_3d.tensor, ap_3d.offset,
                       [list(ap_3d.ap[0]), list(ap_3d.ap[1]),
                        list(ap_3d.ap[2]), [0, n_last]])

    with tile.TileContext(nc) as tc, ExitStack() as ctx:
        const = ctx.enter_context(tc.tile_pool(name="const", bufs=1))
        work = ctx.enter_context(tc.tile_pool(name="work", bufs=2))
        ps_setup = ctx.enter_context(tc.tile_pool(name="ps_setup", bufs=1, space="PSUM"))
        ps_loop = ctx.enter_context(tc.tile_pool(name="ps_loop", bufs=1, space="PSUM"))

        # ---- load constants / weights ----
        tok_sb = const.tile([128, NG * L], i32)
        nc.gpsimd.dma_start(out=tok_sb[:], in_=tok_d)
        consts = const.tile([128, CW], f32)
        nc.sync.dma_start(out=r_(consts[:]), in_=consts_d)
        keysT = consts[0:D, 0:M]
        UwT = consts[0:D, 20:120]
        WwT = consts[0:D, 120:220]
        VwT = consts[0:D, 220:320]
        onesD = consts[0:D, 320:321]
        ones1 = consts[0:1, 321:421]
        blk = consts[0:128, 421:425]

        # ---- Vk = Vw @ keys^T (early; only needs weights) ----
        ps_vk = ps_setup.tile([D, M], f32, tag="psws", name="ps_vk")
        mm(ps_vk[:], VwT, keysT)
        Vk = const.tile([D, M], f32)
        nc.scalar.copy(out=Vk[:], in_=ps_vk[:])
        # bf16 ones row for the cheap 1-cycle/col broadcast matmuls
        bf16 = mybir.dt.bfloat16
        ones1b = const.tile([1, 100], bf16)
        nc.scalar.copy(out=ones1b[:], in_=ones1)

        # ---- gather machinery ----
        # group g covers steps 16g..16g+15 = sentences 128g..128g+127
        # (t-major: sentence j = 8t+b). gather c covers sentences 4c..4c+3:
        # 128 tokens, one per partition; word-sum via f32r matmul with the
        # block-ones matrix into the group psum (D-major, cols = sentence).
        gpool = ctx.enter_context(tc.tile_pool(name="gath", bufs=8))
        sentsD = [const.tile([D, 128], f32, name=f"sentsD{g}", tag=f"sD{g % 2}")
                  for g in range(NG)]
        Ws = [const.tile([D, 128], f32, name=f"ws{g}", tag=f"ws{g % 2}")
              for g in range(NG)]
        ekg = [const.tile([1, GSTEP * R], f32, name=f"ekg{g}",
                          tag=f"ekg{g % 2}") for g in range(NG)]
        vw = [const.tile([D, GSTEP * R], f32, name=f"vw{g}", tag=f"vw{g % 2}")
              for g in range(NG)]
        ps_grp = [ps_setup.tile([D, 128], f32, tag="psgrp", name=f"psgrp{g}")
                  for g in range(NG)]

        def emit_gather(c):
            g = c // GPC
            j = c % GPC
            gt = gpool.tile([128, D], f32, tag="g", name=f"g{c}")
            nc.gpsimd.indirect_dma_start(
                out=r_(gt[:]),
                out_offset=None,
                in_=emb_d,
                in_offset=bass.IndirectOffsetOnAxis(
                    ap=tok_sb[:, c:c + 1], axis=0),
            )
            mm(ps_grp[g][:, 4 * j:4 * j + 4], gt[:], blk)

        psws = [ps_setup.tile([D, 128], f32, tag="psws", name=f"psws{g}")
                for g in range(NG)]

        def emit_qfinalize(g, q):
            # quarter-granular: only depends on gathers 8q..8q+8 of group g
            sl = slice(32 * q, 32 * q + 32)
            nc.scalar.copy(out=r_(sentsD[g][:, sl]), in_=ps_grp[g][:, sl])
            mm(psws[g][:, sl], WwT, sentsD[g][:, sl])
            nc.scalar.copy(out=Ws[g][:, sl], in_=psws[g][:, sl])

        def emit_chunk(g, c):
            # ksraw chunk: keys.s products for steps 4c..4c+3, reduced to
            # kg rows by two [1,320] f32r matmuls, then ekg = exp(-kg)
            ksr = work.tile([D, 4 * R], f32, tag="ksr", name=f"ksr{g}_{c}")
            kv = bass.AP(ksr.tensor, ksr.offset,
                         [list(ksr[:].ap[0]), [R, 4], [M, BL], [1, M]])
            s_sl = sentsD[g][:, 32 * c:32 * c + 32]
            s3 = bass.AP(s_sl.tensor, s_sl.offset,
                         [list(s_sl.ap[0]), [8, 4], [1, 8]])
            nc.vector.tensor_tensor(out=kv.bitcast(f32r),
                                    in0=bcast_mid2(keysT, 4, BL),
                                    in1=bcast_last3(s3, M), op=OP.mult)
            ksf = ksr[:]
            for h in range(2):
                pskg = ps_loop.tile([1, 2 * R], f32, tag="pskg",
                                    name=f"pskg{g}_{c}_{h}")
                mm(pskg[:], onesD, ksf[:, 2 * R * h:2 * R * (h + 1)])
                nc.scalar.activation(
                    out=ekg[g][:, 4 * c * R + 2 * R * h:4 * c * R + 2 * R * (h + 1)],
                    in_=pskg[:], func=AF.Exp, scale=-1.0)
            # vw chunk: Vk + Ws broadcast, [D, 4*160] view [D, 4, 8, 20]
            vsl = vw[g][:, 4 * c * R:(4 * c + 4) * R]
            out = bass.AP(vsl.tensor, vsl.offset,
                          [list(vsl.ap[0]), [R, 4], [M, BL], [1, M]])
            ws_sl = Ws[g][:, 32 * c:32 * c + 32]
            ws3 = bass.AP(ws_sl.tensor, ws_sl.offset,
                          [list(ws_sl.ap[0]), [8, 4], [1, 8]])
            nc.vector.tensor_tensor(out=out, in0=bcast_mid2(Vk[:], 4, BL),
                                    in1=bcast_last3(ws3, M), op=OP.add)

        # ---- lead-in: group 0 gathers + postproc + chunks; group 1 streams
        # during window 0 ----
        for c in range(GPC):
            emit_gather(c)
        for q in range(4):
            emit_qfinalize(0, q)
            emit_chunk(0, q)
        gcur = [GPC]                        # global gather cursor (group 0 done)

        # ---- initial state ----
        U = work.tile([D, R], f32, tag="U")
        nc.vector.tensor_copy(out=r_(U[:].rearrange("d (b m) -> d b m", m=M)),
                              in_=bcast_mid(keysT, BL))
        rho = None
        vkwsn = None

        # ---- recurrence ----
        for t in range(n_steps):
            g, k = t // GSTEP, t % GSTEP
            rs = (t % RESCALE == 0)           # U exactly normalized (rho=n=1)
            rescale_now = ((t + 1) % RESCALE == 0)

            # stream remaining gathers continuously (4 per step, global
            # cursor); quarter-finalize/chunks pipelined one window ahead
            for _ in range(4):
                if gcur[0] < NG * GPC:
                    emit_gather(gcur[0])
                    gcur[0] += 1
            gn = g + 1
            if gn < NG and k in (3, 5, 7, 9):
                q = (k - 3) // 2
                emit_qfinalize(gn, q)
                emit_chunk(gn, q)

            # cand (n-scaled): psA = Uw@U; candf = psA + vw*n
            psA = ps_loop.tile([D, R], f32, tag="psA")
            mm(psA[:], UwT, U[:])

            # gate reduce: psmg = 1^T (U . s_t)
            mgt = work.tile([D, BL, M], f32, tag="mgt")
            nc.vector.tensor_tensor(
                out=r_(mgt[:]),
                in0=U[:].rearrange("d (b m) -> d b m", m=M),
                in1=bcast_last(sentsD[g][:, BL * k:BL * (k + 1)], M),
                op=OP.mult)
            psmg = ps_loop.tile([1, R], f32, tag="psmg")
            mm(psmg[:], onesD, mgt[:].rearrange("d b m -> d (b m)"))

            if rs:
                l_ap = psmg[:]
            else:
                l_sb = work.tile([1, R], f32, tag="l")
                nc.vector.tensor_tensor(out=l_sb[:], in0=psmg[:], in1=rho[:],
                                        op=OP.mult)
                l_ap = l_sb[:]
            e1 = work.tile([1, R], f32, tag="e1")
            nc.scalar.activation(out=e1[:], in_=l_ap, func=AF.Exp, scale=-1.0)
            e_sb = work.tile([1, R], bf16, tag="e")
            nc.vector.tensor_tensor(out=e_sb[:], in0=e1[:],
                                    in1=ekg[g][:, k * R:(k + 1) * R], op=OP.mult)

            # candf = psA + vw_t * n  (vkwsn from prev tail; raw vw when n=1)
            candf = work.tile([D, R], f32, tag="candf")
            if rs:
                nc.vector.tensor_tensor(out=candf[:], in0=psA[:],
                                        in1=vw[g][:, k * R:(k + 1) * R],
                                        op=OP.add)
            else:
                nc.vector.tensor_tensor(out=candf[:], in0=psA[:],
                                        in1=vkwsn, op=OP.add)

            # U' = (1 + e) . U + candf
            psbce = ps_loop.tile([D, R], f32, tag="psbce")
            nc.tensor.matmul(out=psbce[:], lhsT=ones1b[:], rhs=e_sb[:],
                             start=True, stop=True)
            V_sb = work.tile([D, R], f32, tag="V")
            nc.vector.scalar_tensor_tensor(out=V_sb[:], in0=psbce[:],
                                           scalar=1.0, in1=U[:],
                                           op0=OP.add, op1=OP.mult)
            U2 = work.tile([D, R], f32, tag="U")
            nc.vector.tensor_tensor(out=r_(U2[:]), in0=V_sb[:], in1=candf[:],
                                    op=OP.add)
            U = U2

            # norms: rho' = exp(-0.5 ln ss), n' = ss * rho'
            sq = work.tile([D, R], f32, tag="sq")
            nc.vector.tensor_tensor(out=r_(sq[:]), in0=U[:], in1=U[:],
                                    op=OP.mult)
            psss = ps_loop.tile([1, R], f32, tag="psss")
            mm(psss[:], onesD, sq[:])
            lnss = work.tile([1, R], f32, tag="lnss")
            nc.scalar.activation(out=lnss[:], in_=psss[:], func=AF.Ln)
            rho2 = work.tile([1, R], f32, tag="rho")
            nc.scalar.activation(out=r_(rho2[:]), in_=lnss[:], func=AF.Exp,
                                 scale=-0.5)
            rho = rho2

            if rescale_now:
                # exact renormalization: U *= bc(rho); afterwards rho = n = 1
                psbcr = ps_loop.tile([D, R], f32, tag="psbcn", name=f"psbcr{t}")
                mm(psbcr[:], ones1, rho[:])
                U3 = work.tile([D, R], f32, tag="U")
                nc.vector.tensor_tensor(out=r_(U3[:]), in0=psbcr[:], in1=U[:],
                                        op=OP.mult)
                U = U3
            elif t < n_steps - 1:
                # n' for next step's vkwsn
                n_sb = work.tile([1, R], bf16, tag="n")
                nc.vector.tensor_tensor(out=n_sb[:], in0=psss[:], in1=rho[:],
                                        op=OP.mult)
                psbcn = ps_loop.tile([D, R], f32, tag="psbcn")
                nc.tensor.matmul(out=psbcn[:], lhsT=ones1b[:], rhs=n_sb[:],
                                 start=True, stop=True)
                tn = t + 1
                gg, kk = tn // GSTEP, tn % GSTEP
                vkwsn2 = work.tile([D, R], f32, tag="vkwsn")
                nc.vector.tensor_tensor(out=vkwsn2[:], in0=psbcn[:],
                                        in1=vw[gg][:, kk * R:(kk + 1) * R],
                                        op=OP.mult)
                vkwsn = vkwsn2[:]

        # ---- output ----
        if n_steps % RESCALE == 0:
            nc.sync.dma_start(out=out_d, in_=U[:])
        else:
            psbcr = ps_loop.tile([D, R], f32, tag="psbcn")
            mm(psbcr[:], ones1, rho[:])
            memT = work.tile([D, R], f32, tag="memT")
            nc.vector.tensor_tensor(out=memT[:], in0=psbcr[:], in1=U[:],
                                    op=OP.mult)
            nc.sync.dma_start(out=out_d, in_=memT[:])

    _strip_redundant_self_waits(nc)
    return nc


def _np_fallback(tokens, emb, keys, mult, Uw, Vw, Ww, prelu_a):
    tokens = np.asarray(tokens)
    emb = np.asarray(emb, np.float32)
    keys = np.asarray(keys, np.float32)
    mult = np.asarray(mult, np.float32)
    Uw, Vw, Ww = (np.asarray(x, np.float32) for x in (Uw, Vw, Ww))
    a = float(np.asarray(prelu_a).reshape(-1)[0])
    x = emb[tokens] * mult
    sents = x.sum(axis=2, dtype=np.float32)
    mem = np.broadcast_to(keys, (tokens.shape[0], M, D)).astype(np.float32).copy()
    Vk = keys @ Vw.T
    for t in range(tokens.shape[1]):
        s = sents[:, t, :]
        logits = np.einsum('bd,bmd->bm', s, mem) + s @ keys.T
        gate = 1.0 / (1.0 + np.exp(-logits))
        pre = mem @ Uw.T + Vk + (s @ Ww.T)[:, None, :]
        cand = np.where(pre >= 0, pre, a * pre)
        mask = np.where(logits == 0.0, 0.0, 1.0)
        mem = mem + cand * (gate * mask)[:, :, None]
        mem = mem / np.linalg.norm(mem, axis=2, keepdims=True)
    return mem.astype(np.float32)


def _stage_inputs(tokens, emb, keys, mult, Uw, Vw, Ww, prelu_a):
    """Host-side sharding/layout prep. Returns (in_maps, flags)."""
    tokens = np.asarray(tokens)
    emb = np.asarray(emb, dtype=np.float32)
    keys = np.asarray(keys, dtype=np.float32)
    mult = np.asarray(mult, dtype=np.float32)
    a = float(np.asarray(prelu_a).reshape(-1)[0])
    a_is_one = (a == 1.0)
    mult_is_ones = bool(np.all(mult == 1.0))

    emb = np.ascontiguousarray(emb)

    CW = 549
    consts = np.zeros((128, CW), np.float32)
    consts[0:D, 0:M] = keys.T
    consts[0:D, 20:120] = np.asarray(Uw, np.float32).T        # lhsT for Uw@mem
    consts[0:D, 120:220] = np.asarray(Ww, np.float32).T
    consts[0:D, 220:320] = np.asarray(Vw, np.float32).T
    consts[0:D, 320:321] = 1.0                                # onesD
    consts[0:1, 321:421] = 1.0                                # ones1
    consts[0:128, 421:425] = np.kron(np.eye(4, dtype=np.float32),
                                     np.ones((L, 1), np.float32))  # blk

    in_maps = []
    for c in range(NCORES):
        tc_ = tokens[c * BL:(c + 1) * BL]                     # [8, S, L]
        # sentence-major rows with t-major sentence order: row j = 8t+b
        tokflat = np.ascontiguousarray(tc_.transpose(1, 0, 2)).reshape(NS, L)
        # tok_staged[p, c] = token of sentence 4c + p//32, word p%32
        tok_staged = np.ascontiguousarray(
            tokflat.reshape(2 * S, 4, L).transpose(1, 2, 0)).reshape(128, 2 * S)
        in_maps.append({"tok": np.ascontiguousarray(tok_staged, np.int32),
                        "emb": emb, "consts": consts})
    return in_maps, a_is_one, mult_is_ones, a


def kernel(tokens, emb, keys, mult, Uw, Vw, Ww, prelu_a, _trace=False):
    from concourse.bass_utils import run_bass_kernel_spmd

    in_maps, a_is_one, mult_is_ones, a = _stage_inputs(
        tokens, emb, keys, mult, Uw, Vw, Ww, prelu_a)
    if not (a_is_one and mult_is_ones):
        return _np_fallback(tokens, emb, keys, mult, Uw, Vw, Ww, prelu_a)

    key = (a_is_one, mult_is_ones)
    if key not in _prog_cache:
        _prog_cache[key] = _build_program(a_is_one, mult_is_ones, a)
    nc = _prog_cache[key]

    res = run_bass_kernel_spmd(nc, in_maps, list(range(NCORES)), trace=_trace)
    out = np.empty((B, M, D), dtype=np.float32)
    for c in range(NCORES):
        memT = res.results[c]["memT"]                          # [D, R]
        out[c * BL:(c + 1) * BL] = memT.reshape(D, BL, M).transpose(1, 2, 0)
    kernel._last_results = res
    return out


# revision 14
# speedup vs baseline: 1.2363x; 1.0384x over previous
"""EntNet Trainium2 kernel (8-core data-parallel over batch).

Reference computation (shapes: B=64, S=128, L=32, D=100, M=20, V=50000):
  sents = (emb[tokens] * mult).sum(axis=2)            # [B,S,D]
  mem0 = broadcast(keys)                              # [B,M,D]
  per step t: gate = sigmoid(s.mem + s.keys); cand = prelu(mem@Uw.T + keys@Vw.T + s@Ww.T)
              mem = normalize(mem + cand*gate, axis=D)

Kernel strategy per core (8 batches/core, R = 8*20 = 160 (b,m) rows):
  - Embedding gather via gpsimd indirect DMA (128 tokens = 4 sentences per
    instruction), word-summed by one tiny f32r matmul per gather straight
    into a per-group D-major psum [100,128] (Tile's scheduler models bypass
    gathers accurately, so it interleaves the stream without queue stalls;
    accumulating gathers run 60% over model and head-of-line block queues).
  - Key-gate logits kg = s.keys precomputed per group from sents via
    keys-broadcast DVE products reduced by [1,320] f32r matmuls.
  - Recurrence kept in scale-free form: with U unnormalized and
    rho = 1/||U|| per row, the update
        mem' = normalize(mem + cand * sigmoid(l))
    is exactly
        U' = (1 + exp(-l)) . U + Uw@U + (Vk + Ws_t) * n        (n = ||U||)
    Gate split: exp(-l) = exp(-rho * 1^T(U.s)) * exp(-kg), with exp(-kg)
    precomputed per 16-step group (kg rows extracted by a SBUF->SBUF DMA
    transpose-gather), so no key-term matmul and no n-scaled keys on the
    critical path.
  - All matmuls run as float32r (single-pass PE mode) instead of float32
    (which costs 2 half-rate passes + 2 LDWEIGHTS each).
"""

import numpy as np

B, S, L, D, M, V = 64, 128, 32, 100, 20, 50000
NCORES = 8
BL = B // NCORES            # 8 batches per core
NS = BL * S                 # 1024 sentences per core
R = BL * M                  # 160 (b, m) rows per core
NG = 8                      # gather groups (16 steps each)
GSTEP = S // NG             # 16 steps per group
GPC = 32                    # gathers per group (4 sentences each)
RESCALE = 8                 # renormalize U every RESCALE steps (f32 range)

_prog_cache = {}

_ENGINE_SEM = {"PE": "PE_", "DVE": "DVE_", "Activation": "Activation_",
               "Pool": "Pool_", "SP": "SP_"}


def _strip_redundant_self_waits(nc):
    """Legalize sync waits: walrus rejects >1 sync wait on most instruction
    structs. For any instruction carrying several, hoist all but one onto
    preceding single-wait NoOps on the same engine queue (in-order dispatch
    keeps semantics). The instruction keeps its OWN-engine wait if it has one
    (that wait guards an engine-pipelining RAW hazard and must gate execution,
    not just dispatch).
    """
    import concourse.mybir as mybir
    for fn in nc.m.functions:
        for blk in fn.blocks:
            i = 0
            while i < len(blk.instructions):
                inst = blk.instructions[i]
                si = inst.sync_info() if callable(inst.sync_info) else inst.sync_info
                if (si is not None and si.on_wait and len(si.on_wait) > 1
                        and inst.engine is not None):
                    waits = list(si.on_wait)
                    pref = _ENGINE_SEM.get(inst.engine.name)
                    keep_idx = None
                    for j, w in enumerate(waits):
                        if pref and w.ant_name.startswith(pref):
                            keep_idx = j
                            break
                    kept = [waits.pop(keep_idx)] if keep_idx is not None else []
                    noops = []
                    for w in waits:
                        nop = mybir.InstNoOp(
                            name=nc.get_next_instruction_name(), ins=[], outs=[])
                        nop.engine = inst.engine
                        nop.sync_info = mybir.SyncInfo(on_wait=[w], on_update=[])
                        nc.register_instruction(nop, overwrite=True)
                        noops.append(nop)
                    inst.sync_info = mybir.SyncInfo(
                        on_wait=kept, on_update=list(si.on_update))
                    blk.instructions[i:i] = noops
                    i += len(noops)
                i += 1


def _build_program(a_is_one: bool, mult_is_ones: bool, alpha: float,
                   n_steps: int = S, dump: bool = False):
    import concourse.bass as bass
    import concourse.tile as tile
    from concourse import mybir
    from contextlib import ExitStack

    assert mult_is_ones and a_is_one, "fast path only (host fallback otherwise)"

    f32 = mybir.dt.float32
    f32r = mybir.dt.float32r
    i32 = mybir.dt.int32
    AF = mybir.ActivationFunctionType
    OP = mybir.AluOpType

    nc = bass.Bass(trn_type="TRN2")

    # ---- DRAM I/O ----
    CW = 549
    tok_d = nc.dram_tensor("tok", [128, NG * L], i32, kind="ExternalInput").ap()
    emb_d = nc.dram_tensor("emb", [V, D], f32r, kind="ExternalInput").ap()
    consts_d = nc.dram_tensor("consts", [128, CW], f32r, kind="ExternalInput").ap()
    out_d = nc.dram_tensor("memT", [D, R], f32, kind="ExternalOutput").ap()

    def r_(ap):
        # f32r = fp32 bits fed to the PE in single-pass (replicated) mode.
        # walrus's BIR verifier requires producers of f32r-matmul operands to
        # declare f32r-rounded output, so producer OUT aps are bitcast too.
        return ap.bitcast(f32r)

    def mm(out, lhsT, rhs, start=True, stop=True):
        nc.tensor.matmul(out=out, lhsT=r_(lhsT), rhs=r_(rhs),
                         start=start, stop=stop)

    def bcast_mid(ap_2d, n_mid):
        # [P, k] -> [P, n_mid, k] with stride-0 middle dim
        return bass.AP(ap_2d.tensor, ap_2d.offset,
                       [list(ap_2d.ap[0]), [0, n_mid], list(ap_2d.ap[1])])

    def bcast_last(ap_2d, n_last):
        # [P, k] -> [P, k, n_last] with stride-0 last dim
        return bass.AP(ap_2d.tensor, ap_2d.offset,
                       [list(ap_2d.ap[0]), list(ap_2d.ap[1]), [0, n_last]])

    def bcast_mid2(ap_2d, n1, n2):
        # [P, k] -> [P, n1, n2, k] with stride-0 middle dims
        return bass.AP(ap_2d.tensor, ap_2d.offset,
                       [list(ap_2d.ap[0]), [0, n1], [0, n2], list(ap_2d.ap[1])])

    def bcast_last3(ap_3d, n_last):
        # [P, a, b] -> [P, a, b, n_last] with stride-0 last dim
        return bass.AP(ap_3d.tensor, ap_3d.offset,
                       [list(ap_3d.ap[0]), list(ap_3d.ap[1]),
                        list(ap_3d.ap[2]), [0, n_last]])

    with tile.TileContext(nc) as tc, ExitStack() as ctx:
        const = ctx.enter_context(tc.tile_pool(name="const", bufs=1))
        work = ctx.enter_context(tc.tile_pool(name="work", bufs=2))
        ps_setup = ctx.enter_context(tc.tile_pool(name="ps_setup", bufs=1, space="PSUM"))
        ps_loop = ctx.enter_context(tc.tile_pool(name="ps_loop", bufs=1, space="PSUM"))

        # ---- load constants / weights ----
        tok_sb = const.tile([128, NG * L], i32)
        nc.gpsimd.dma_start(out=tok_sb[:], in_=tok_d)
        consts = const.tile([128, CW], f32)
        nc.sync.dma_start(out=r_(consts[:]), in_=consts_d)
        keysT = consts[0:D, 0:M]
        UwT = consts[0:D, 20:120]
        WwT = consts[0:D, 120:220]
        VwT = consts[0:D, 220:320]
        onesD = consts[0:D, 320:321]
        ones1 = consts[0:1, 321:421]
        blk = consts[0:128, 421:425]

        # ---- Vk = Vw @ keys^T (early; only needs weights) ----
        ps_vk = ps_setup.tile([D, M], f32, tag="psws", name="ps_vk")
        mm(ps_vk[:], VwT, keysT)
        Vk = const.tile([D, M], f32)
        nc.scalar.copy(out=Vk[:], in_=ps_vk[:])
        # bf16 ones row for the cheap 1-cycle/col broadcast matmuls
        bf16 = mybir.dt.bfloat16
        ones1b = const.tile([1, 100], bf16)
        nc.scalar.copy(out=ones1b[:], in_=ones1)

        # ---- gather machinery ----
        # group g covers steps 16g..16g+15 = sentences 128g..128g+127
        # (t-major: sentence j = 8t+b). gather c covers sentences 4c..4c+3:
        # 128 tokens, one per partition; word-sum via f32r matmul with the
        # block-ones matrix into the group psum (D-major, cols = sentence).
        gpool = ctx.enter_context(tc.tile_pool(name="gath", bufs=16))
        sentsD = [const.tile([D, 128], f32, name=f"sentsD{g}", tag=f"sD{g % 2}")
                  for g in range(NG)]
        Ws = [const.tile([D, 128], f32, name=f"ws{g}", tag=f"ws{g % 2}")
              for g in range(NG)]
        ekg = [const.tile([1, GSTEP * R], f32, name=f"ekg{g}",
                          tag=f"ekg{g % 2}") for g in range(NG)]
        vw = [const.tile([D, GSTEP * R], f32, name=f"vw{g}", tag=f"vw{g % 2}")
              for g in range(NG)]
        ps_grp = [ps_setup.tile([D, 128], f32, tag="psgrp", name=f"psgrp{g}")
                  for g in range(NG)]

        gtiles = {}

        def emit_dma(c):
            gt = gpool.tile([128, D], f32, tag="g", name=f"g{c}")
            gtiles[c] = gt
            nc.gpsimd.indirect_dma_start(
                out=r_(gt[:]),
                out_offset=None,
                in_=emb_d,
                in_offset=bass.IndirectOffsetOnAxis(
                    ap=tok_sb[:, c:c + 1], axis=0),
            )

        def emit_mm1(c):
            g = c // GPC
            j = c % GPC
            gt = gtiles.pop(c)
            mm(ps_grp[g][:, 4 * j:4 * j + 4], gt[:], blk)

        MMLAG = 8

        def emit_gather(c):
            # DMA for gather c; reduce-matmul for gather c-MMLAG. The lag
            # keeps per-gather matmuls queued on PE well after their DMA has
            # landed, so the in-order PE queue never blocks on the Pool-bound
            # gather stream.
            emit_dma(c)
            if c >= MMLAG:
                emit_mm1(c - MMLAG)

        def flush_mm1():
            for c in range(NG * GPC - MMLAG, NG * GPC):
                emit_mm1(c)

        psws = [ps_setup.tile([D, 128], f32, tag="psws", name=f"psws{g}")
                for g in range(NG)]

        def emit_qfinalize(g, q):
            # quarter-granular: only depends on gathers 8q..8q+8 of group g
            sl = slice(32 * q, 32 * q + 32)
            nc.scalar.copy(out=r_(sentsD[g][:, sl]), in_=ps_grp[g][:, sl])
            mm(psws[g][:, sl], WwT, sentsD[g][:, sl])
            nc.scalar.copy(out=Ws[g][:, sl], in_=psws[g][:, sl])

        def emit_chunk(g, c):
            # ksraw chunk: keys.s products for steps 4c..4c+3, reduced to
            # kg rows by two [1,320] f32r matmuls, then ekg = exp(-kg)
            ksr = work.tile([D, 4 * R], f32, tag="ksr", name=f"ksr{g}_{c}")
            kv = bass.AP(ksr.tensor, ksr.offset,
                         [list(ksr[:].ap[0]), [R, 4], [M, BL], [1, M]])
            s_sl = sentsD[g][:, 32 * c:32 * c + 32]
            s3 = bass.AP(s_sl.tensor, s_sl.offset,
                         [list(s_sl.ap[0]), [8, 4], [1, 8]])
            nc.vector.tensor_tensor(out=kv.bitcast(f32r),
                                    in0=bcast_mid2(keysT, 4, BL),
                                    in1=bcast_last3(s3, M), op=OP.mult)
            ksf = ksr[:]
            for h in range(2):
                pskg = ps_loop.tile([1, 2 * R], f32, tag="pskg",
                                    name=f"pskg{g}_{c}_{h}")
                mm(pskg[:], onesD, ksf[:, 2 * R * h:2 * R * (h + 1)])
                nc.scalar.activation(
                    out=ekg[g][:, 4 * c * R + 2 * R * h:4 * c * R + 2 * R * (h + 1)],
                    in_=pskg[:], func=AF.Exp, scale=-1.0)
            # vw chunk: Vk + Ws broadcast, [D, 4*160] view [D, 4, 8, 20]
            vsl = vw[g][:, 4 * c * R:(4 * c + 4) * R]
            out = bass.AP(vsl.tensor, vsl.offset,
                          [list(vsl.ap[0]), [R, 4], [M, BL], [1, M]])
            ws_sl = Ws[g][:, 32 * c:32 * c + 32]
            ws3 = bass.AP(ws_sl.tensor, ws_sl.offset,
                          [list(ws_sl.ap[0]), [8, 4], [1, 8]])
            nc.vector.tensor_tensor(out=out, in0=bcast_mid2(Vk[:], 4, BL),
                                    in1=bcast_last3(ws3, M), op=OP.add)

        # ---- lead-in: group 0 gathers + postproc + chunks; group 1 streams
        # during window 0 ----
        for c in range(GPC + MMLAG):
            if c < NG * GPC:
                emit_gather(c)
        for q in range(4):
            emit_qfinalize(0, q)
            emit_chunk(0, q)
        gcur = [GPC + MMLAG]                # global gather cursor (group 0 done)

        # ---- initial state ----
        U = work.tile([D, R], f32, tag="U")
        nc.vector.tensor_copy(out=r_(U[:].rearrange("d (b m) -> d b m", m=M)),
                              in_=bcast_mid(keysT, BL))
        rho = None
        vkwsn = None

        # ---- recurrence ----
        for t in range(n_steps):
            g, k = t // GSTEP, t % GSTEP
            rs = (t % RESCALE == 0)           # U exactly normalized (rho=n=1)
            rescale_now = ((t + 1) % RESCALE == 0)

            # stream remaining gathers continuously (4 per step, global
            # cursor); quarter-finalize/chunks pipelined one window ahead
            for _ in range(4):
                if gcur[0] < NG * GPC:
                    emit_gather(gcur[0])
                    gcur[0] += 1
                elif gcur[0] == NG * GPC:
                    flush_mm1()
                    gcur[0] += 1
            gn = g + 1
            if gn < NG and k in (3, 5, 7, 9):
                q = (k - 3) // 2
                emit_qfinalize(gn, q)
                emit_chunk(gn, q)

            # cand (n-scaled): psA = Uw@U; candf = psA + vw*n
            psA = ps_loop.tile([D, R], f32, tag="psA")
            mm(psA[:], UwT, U[:])

            # gate reduce: psmg = 1^T (U . s_t)
            mgt = work.tile([D, BL, M], f32, tag="mgt")
            nc.vector.tensor_tensor(
                out=r_(mgt[:]),
                in0=U[:].rearrange("d (b m) -> d b m", m=M),
                in1=bcast_last(sentsD[g][:, BL * k:BL * (k + 1)], M),
                op=OP.mult)
            psmg = ps_loop.tile([1, R], f32, tag="psmg")
            mm(psmg[:], onesD, mgt[:].rearrange("d b m -> d (b m)"))

            if rs:
                l_ap = psmg[:]
            else:
                l_sb = work.tile([1, R], f32, tag="l")
                nc.vector.tensor_tensor(out=l_sb[:], in0=psmg[:], in1=rho[:],
                                        op=OP.mult)
                l_ap = l_sb[:]
            e1 = work.tile([1, R], f32, tag="e1")
            nc.scalar.activation(out=e1[:], in_=l_ap, func=AF.Exp, scale=-1.0)
            e_sb = work.tile([1, R], bf16, tag="e")
            nc.vector.tensor_tensor(out=e_sb[:], in0=e1[:],
                                    in1=ekg[g][:, k * R:(k + 1) * R], op=OP.mult)

            # candf = psA + vw_t * n  (vkwsn from prev tail; raw vw when n=1)
            candf = work.tile([D, R], f32, tag="candf")
            if rs:
                nc.vector.tensor_tensor(out=candf[:], in0=psA[:],
                                        in1=vw[g][:, k * R:(k + 1) * R],
                                        op=OP.add)
            else:
                nc.vector.tensor_tensor(out=candf[:], in0=psA[:],
                                        in1=vkwsn, op=OP.add)

            # U' = (1 + e) . U + candf
            psbce = ps_loop.tile([D, R], f32, tag="psbce")
            nc.tensor.matmul(out=psbce[:], lhsT=ones1b[:], rhs=e_sb[:],
                             start=True, stop=True)
            V_sb = work.tile([D, R], f32, tag="V")
            nc.vector.scalar_tensor_tensor(out=V_sb[:], in0=psbce[:],
                                           scalar=1.0, in1=U[:],
                                           op0=OP.add, op1=OP.mult)
            U2 = work.tile([D, R], f32, tag="U")
            nc.vector.tensor_tensor(out=r_(U2[:]), in0=V_sb[:], in1=candf[:],
                                    op=OP.add)
            U = U2

            # norms: rho' = exp(-0.5 ln ss), n' = ss * rho'
            sq = work.tile([D, R], f32, tag="sq")
            nc.vector.tensor_tensor(out=r_(sq[:]), in0=U[:], in1=U[:],
                                    op=OP.mult)
            psss = ps_loop.tile([1, R], f32, tag="psss")
            mm(psss[:], onesD, sq[:])
            lnss = work.tile([1, R], f32, tag="lnss")
            nc.scalar.activation(out=lnss[:], in_=psss[:], func=AF.Ln)
            rho2 = work.tile([1, R], f32, tag="rho")
            nc.scalar.activation(out=r_(rho2[:]), in_=lnss[:], func=AF.Exp,
                                 scale=-0.5)
            rho = rho2

            if rescale_now:
                # exact renormalization: U *= bc(rho); afterwards rho = n = 1
                psbcr = ps_loop.tile([D, R], f32, tag="psbcn", name=f"psbcr{t}")
                mm(psbcr[:], ones1, rho[:])
                U3 = work.tile([D, R], f32, tag="U")
                nc.vector.tensor_tensor(out=r_(U3[:]), in0=psbcr[:], in1=U[:],
                                        op=OP.mult)
                U = U3
            elif t < n_steps - 1:
                # n' for next step's vkwsn
                n_sb = work.tile([1, R], bf16, tag="n")
                nc.vector.tensor_tensor(out=n_sb[:], in0=psss[:], in1=rho[:],
                                        op=OP.mult)
                psbcn = ps_loop.tile([D, R], f32, tag="psbcn")
                nc.tensor.matmul(out=psbcn[:], lhsT=ones1b[:], rhs=n_sb[:],
                                 start=True, stop=True)
                tn = t + 1
                gg, kk = tn // GSTEP, tn % GSTEP
                vkwsn2 = work.tile([D, R], f32, tag="vkwsn")
                nc.vector.tensor_tensor(out=vkwsn2[:], in0=psbcn[:],
                                        in1=vw[gg][:, kk * R:(kk + 1) * R],
                                        op=OP.mult)
                vkwsn = vkwsn2[:]

        # ---- output ----
        if n_steps % RESCALE == 0:
            nc.sync.dma_start(out=out_d, in_=U[:])
        else:
            psbcr = ps_loop.tile([D, R], f32, tag="psbcn")
            mm(psbcr[:], ones1, rho[:])
            memT = work.tile([D, R], f32, tag="memT")
            nc.vector.tensor_tensor(out=memT[:], in0=psbcr[:], in1=U[:],
                                    op=OP.mult)
            nc.sync.dma_start(out=out_d, in_=memT[:])

    _strip_redundant_self_waits(nc)
    return nc


def _np_fallback(tokens, emb, keys, mult, Uw, Vw, Ww, prelu_a):
    tokens = np.asarray(tokens)
    emb = np.asarray(emb, np.float32)
    keys = np.asarray(keys, np.float32)
    mult = np.asarray(mult, np.float32)
    Uw, Vw, Ww = (np.asarray(x, np.float32) for x in (Uw, Vw, Ww))
    a = float(np.asarray(prelu_a).reshape(-1)[0])
    x = emb[tokens] * mult
    sents = x.sum(axis=2, dtype=np.float32)
    mem = np.broadcast_to(keys, (tokens.shape[0], M, D)).astype(np.float32).copy()
    Vk = keys @ Vw.T
    for t in range(tokens.shape[1]):
        s = sents[:, t, :]
        logits = np.einsum('bd,bmd->bm', s, mem) + s @ keys.T
        gate = 1.0 / (1.0 + np.exp(-logits))
        pre = mem @ Uw.T + Vk + (s @ Ww.T)[:, None, :]
        cand = np.where(pre >= 0, pre, a * pre)
        mask = np.where(logits == 0.0, 0.0, 1.0)
        mem = mem + cand * (gate * mask)[:, :, None]
        mem = mem / np.linalg.norm(mem, axis=2, keepdims=True)
    return mem.astype(np.float32)


def _stage_inputs(tokens, emb, keys, mult, Uw, Vw, Ww, prelu_a):
    """Host-side sharding/layout prep. Returns (in_maps, flags)."""
    tokens = np.asarray(tokens)
    emb = np.asarray(emb, dtype=np.float32)
    keys = np.asarray(keys, dtype=np.float32)
    mult = np.asarray(mult, dtype=np.float32)
    a = float(np.asarray(prelu_a).reshape(-1)[0])
    a_is_one = (a == 1.0)
    mult_is_ones = bool(np.all(mult == 1.0))

    emb = np.ascontiguousarray(emb)

    CW = 549
    consts = np.zeros((128, CW), np.float32)
    consts[0:D, 0:M] = keys.T
    consts[0:D, 20:120] = np.asarray(Uw, np.float32).T        # lhsT for Uw@mem
    consts[0:D, 120:220] = np.asarray(Ww, np.float32).T
    consts[0:D, 220:320] = np.asarray(Vw, np.float32).T
    consts[0:D, 320:321] = 1.0                                # onesD
    consts[0:1, 321:421] = 1.0                                # ones1
    consts[0:128, 421:425] = np.kron(np.eye(4, dtype=np.float32),
                                     np.ones((L, 1), np.float32))  # blk

    in_maps = []
    for c in range(NCORES):
        tc_ = tokens[c * BL:(c + 1) * BL]                     # [8, S, L]
        # sentence-major rows with t-major sentence order: row j = 8t+b
        tokflat = np.ascontiguousarray(tc_.transpose(1, 0, 2)).reshape(NS, L)
        # tok_staged[p, c] = token of sentence 4c + p//32, word p%32
        tok_staged = np.ascontiguousarray(
            tokflat.reshape(2 * S, 4, L).transpose(1, 2, 0)).reshape(128, 2 * S)
        in_maps.append({"tok": np.ascontiguousarray(tok_staged, np.int32),
                        "emb": emb, "consts": consts})
    return in_maps, a_is_one, mult_is_ones, a


def kernel(tokens, emb, keys, mult, Uw, Vw, Ww, prelu_a, _trace=False):
    from concourse.bass_utils import run_bass_kernel_spmd

    in_maps, a_is_one, mult_is_ones, a = _stage_inputs(
        tokens, emb, keys, mult, Uw, Vw, Ww, prelu_a)
    if not (a_is_one and mult_is_ones):
        return _np_fallback(tokens, emb, keys, mult, Uw, Vw, Ww, prelu_a)

    key = (a_is_one, mult_is_ones)
    if key not in _prog_cache:
        _prog_cache[key] = _build_program(a_is_one, mult_is_ones, a)
    nc = _prog_cache[key]

    res = run_bass_kernel_spmd(nc, in_maps, list(range(NCORES)), trace=_trace)
    out = np.empty((B, M, D), dtype=np.float32)
    for c in range(NCORES):
        memT = res.results[c]["memT"]                          # [D, R]
        out[c * BL:(c + 1) * BL] = memT.reshape(D, BL, M).transpose(1, 2, 0)
    kernel._last_results = res
    return out


# revision 15
# speedup vs baseline: 1.3170x; 1.0652x over previous
"""EntNet Trainium2 kernel (8-core data-parallel over batch).

Reference computation (shapes: B=64, S=128, L=32, D=100, M=20, V=50000):
  sents = (emb[tokens] * mult).sum(axis=2)            # [B,S,D]
  mem0 = broadcast(keys)                              # [B,M,D]
  per step t: gate = sigmoid(s.mem + s.keys); cand = prelu(mem@Uw.T + keys@Vw.T + s@Ww.T)
              mem = normalize(mem + cand*gate, axis=D)

Kernel strategy per core (8 batches/core, R = 8*20 = 160 (b,m) rows):
  - Embedding gather via gpsimd indirect DMA (128 tokens = 4 sentences per
    instruction), word-summed by one tiny f32r matmul per gather straight
    into a per-group D-major psum [100,128] (Tile's scheduler models bypass
    gathers accurately, so it interleaves the stream without queue stalls;
    accumulating gathers run 60% over model and head-of-line block queues).
  - Key-gate logits kg = s.keys precomputed per group from sents via
    keys-broadcast DVE products reduced by [1,320] f32r matmuls.
  - Recurrence kept in scale-free form: with U unnormalized and
    rho = 1/||U|| per row, the update
        mem' = normalize(mem + cand * sigmoid(l))
    is exactly
        U' = (1 + exp(-l)) . U + Uw@U + (Vk + Ws_t) * n        (n = ||U||)
    Gate split: exp(-l) = exp(-rho * 1^T(U.s)) * exp(-kg), with exp(-kg)
    precomputed per 16-step group (kg rows extracted by a SBUF->SBUF DMA
    transpose-gather), so no key-term matmul and no n-scaled keys on the
    critical path.
  - All matmuls run as float32r (single-pass PE mode) instead of float32
    (which costs 2 half-rate passes + 2 LDWEIGHTS each).
"""

import numpy as np

B, S, L, D, M, V = 64, 128, 32, 100, 20, 50000
NCORES = 8
BL = B // NCORES            # 8 batches per core
NS = BL * S                 # 1024 sentences per core
R = BL * M                  # 160 (b, m) rows per core
NG = 8                      # gather groups (16 steps each)
GSTEP = S // NG             # 16 steps per group
GPC = 32                    # gathers per group (4 sentences each)
RESCALE = 8                 # renormalize U every RESCALE steps (f32 range)

_prog_cache = {}

_ENGINE_SEM = {"PE": "PE_", "DVE": "DVE_", "Activation": "Activation_",
               "Pool": "Pool_", "SP": "SP_"}


def _strip_redundant_self_waits(nc):
    """Legalize sync waits: walrus rejects >1 sync wait on most instruction
    structs. For any instruction carrying several, hoist all but one onto
    preceding single-wait NoOps on the same engine queue (in-order dispatch
    keeps semantics). The instruction keeps its OWN-engine wait if it has one
    (that wait guards an engine-pipelining RAW hazard and must gate execution,
    not just dispatch).
    """
    import concourse.mybir as mybir
    for fn in nc.m.functions:
        for blk in fn.blocks:
            i = 0
            while i < len(blk.instructions):
                inst = blk.instructions[i]
                si = inst.sync_info() if callable(inst.sync_info) else inst.sync_info
                if (si is not None and si.on_wait and len(si.on_wait) > 1
                        and inst.engine is not None):
                    waits = list(si.on_wait)
                    pref = _ENGINE_SEM.get(inst.engine.name)
                    keep_idx = None
                    for j, w in enumerate(waits):
                        if pref and w.ant_name.startswith(pref):
                            keep_idx = j
                            break
                    kept = [waits.pop(keep_idx)] if keep_idx is not None else []
                    noops = []
                    for w in waits:
                        nop = mybir.InstNoOp(
                            name=nc.get_next_instruction_name(), ins=[], outs=[])
                        nop.engine = inst.engine
                        nop.sync_info = mybir.SyncInfo(on_wait=[w], on_update=[])
                        nc.register_instruction(nop, overwrite=True)
                        noops.append(nop)
                    inst.sync_info = mybir.SyncInfo(
                        on_wait=kept, on_update=list(si.on_update))
                    blk.instructions[i:i] = noops
                    i += len(noops)
                i += 1


def _build_program(a_is_one: bool, mult_is_ones: bool, alpha: float,
                   n_steps: int = S, dump: bool = False):
    import concourse.bass as bass
    import concourse.tile as tile
    from concourse import mybir
    from contextlib import ExitStack

    assert mult_is_ones and a_is_one, "fast path only (host fallback otherwise)"

    f32 = mybir.dt.float32
    f32r = mybir.dt.float32r
    i32 = mybir.dt.int32
    AF = mybir.ActivationFunctionType
    OP = mybir.AluOpType

    nc = bass.Bass(trn_type="TRN2")

    # ---- DRAM I/O ----
    CW = 549
    tok_d = nc.dram_tensor("tok", [128, NG * L], i32, kind="ExternalInput").ap()
    emb_d = nc.dram_tensor("emb", [V, D], f32r, kind="ExternalInput").ap()
    consts_d = nc.dram_tensor("consts", [128, CW], f32r, kind="ExternalInput").ap()
    out_d = nc.dram_tensor("memT", [D, R], f32, kind="ExternalOutput").ap()

    def r_(ap):
        # f32r = fp32 bits fed to the PE in single-pass (replicated) mode.
        # walrus's BIR verifier requires producers of f32r-matmul operands to
        # declare f32r-rounded output, so producer OUT aps are bitcast too.
        return ap.bitcast(f32r)

    def mm(out, lhsT, rhs, start=True, stop=True):
        nc.tensor.matmul(out=out, lhsT=r_(lhsT), rhs=r_(rhs),
                         start=start, stop=stop)

    def bcast_mid(ap_2d, n_mid):
        # [P, k] -> [P, n_mid, k] with stride-0 middle dim
        return bass.AP(ap_2d.tensor, ap_2d.offset,
                       [list(ap_2d.ap[0]), [0, n_mid], list(ap_2d.ap[1])])

    def bcast_last(ap_2d, n_last):
        # [P, k] -> [P, k, n_last] with stride-0 last dim
        return bass.AP(ap_2d.tensor, ap_2d.offset,
                       [list(ap_2d.ap[0]), list(ap_2d.ap[1]), [0, n_last]])

    def bcast_mid2(ap_2d, n1, n2):
        # [P, k] -> [P, n1, n2, k] with stride-0 middle dims
        return bass.AP(ap_2d.tensor, ap_2d.offset,
                       [list(ap_2d.ap[0]), [0, n1], [0, n2], list(ap_2d.ap[1])])

    def bcast_last3(ap_3d, n_last):
        # [P, a, b] -> [P, a, b, n_last] with stride-0 last dim
        return bass.AP(ap_3d.tensor, ap_3d.offset,
                       [list(ap_3d.ap[0]), list(ap_3d.ap[1]),
                        list(ap_3d.ap[2]), [0, n_last]])

    with tile.TileContext(nc) as tc, ExitStack() as ctx:
        const = ctx.enter_context(tc.tile_pool(name="const", bufs=1))
        work = ctx.enter_context(tc.tile_pool(name="work", bufs=2))
        ps_setup = ctx.enter_context(tc.tile_pool(name="ps_setup", bufs=1, space="PSUM"))
        ps_loop = ctx.enter_context(tc.tile_pool(name="ps_loop", bufs=1, space="PSUM"))

        # ---- load constants / weights ----
        tok_sb = const.tile([128, NG * L], i32)
        nc.gpsimd.dma_start(out=tok_sb[:], in_=tok_d)
        consts = const.tile([128, CW], f32)
        nc.sync.dma_start(out=r_(consts[:]), in_=consts_d)
        keysT = consts[0:D, 0:M]
        UwT = consts[0:D, 20:120]
        WwT = consts[0:D, 120:220]
        VwT = consts[0:D, 220:320]
        onesD = consts[0:D, 320:321]
        ones1 = consts[0:1, 321:421]
        blk = consts[0:128, 421:425]

        # ---- Vk = Vw @ keys^T (early; only needs weights) ----
        ps_vk = ps_setup.tile([D, M], f32, tag="psws", name="ps_vk")
        mm(ps_vk[:], VwT, keysT)
        Vk = const.tile([D, M], f32)
        nc.scalar.copy(out=Vk[:], in_=ps_vk[:])
        # bf16 ones row for the cheap 1-cycle/col broadcast matmuls
        bf16 = mybir.dt.bfloat16
        ones1b = const.tile([1, 100], bf16)
        nc.scalar.copy(out=ones1b[:], in_=ones1)

        # ---- gather machinery ----
        # group g covers steps 16g..16g+15 = sentences 128g..128g+127
        # (t-major: sentence j = 8t+b). gather c covers sentences 4c..4c+3:
        # 128 tokens, one per partition; word-sum via f32r matmul with the
        # block-ones matrix into the group psum (D-major, cols = sentence).
        gpool = ctx.enter_context(tc.tile_pool(name="gath", bufs=64))
        sentsD = [const.tile([D, 128], f32, name=f"sentsD{g}", tag=f"sD{g % 2}")
                  for g in range(NG)]
        Ws = [const.tile([D, 128], f32, name=f"ws{g}", tag=f"ws{g % 2}")
              for g in range(NG)]
        ekg = [const.tile([1, GSTEP * R], f32, name=f"ekg{g}",
                          tag=f"ekg{g % 2}") for g in range(NG)]
        vw = [const.tile([D, GSTEP * R], f32, name=f"vw{g}", tag=f"vw{g % 2}")
              for g in range(NG)]
        ps_grp = [ps_setup.tile([D, 128], f32, tag="psgrp", name=f"psgrp{g}")
                  for g in range(NG)]

        gtiles = {}

        def emit_dma(c):
            gt = gpool.tile([128, D], f32, tag="g", name=f"g{c}")
            gtiles[c] = gt
            nc.gpsimd.indirect_dma_start(
                out=r_(gt[:]),
                out_offset=None,
                in_=emb_d,
                in_offset=bass.IndirectOffsetOnAxis(
                    ap=tok_sb[:, c:c + 1], axis=0),
            )

        def emit_mm1(c):
            g = c // GPC
            j = c % GPC
            gt = gtiles.pop(c)
            mm(ps_grp[g][:, 4 * j:4 * j + 4], gt[:], blk)

        MMLAG = 8

        def emit_gather(c):
            # DMA for gather c; reduce-matmul for gather c-MMLAG. The lag
            # keeps per-gather matmuls queued on PE well after their DMA has
            # landed, so the in-order PE queue never blocks on the Pool-bound
            # gather stream.
            emit_dma(c)
            if c - MMLAG >= GPC:
                emit_mm1(c - MMLAG)

        def flush_mm1():
            for c in range(NG * GPC - MMLAG, NG * GPC):
                emit_mm1(c)

        psws = [ps_setup.tile([D, 128], f32, tag="psws", name=f"psws{g}")
                for g in range(NG)]

        def emit_qfinalize(g, q):
            # quarter-granular: only depends on gathers 8q..8q+8 of group g
            sl = slice(32 * q, 32 * q + 32)
            nc.scalar.copy(out=r_(sentsD[g][:, sl]), in_=ps_grp[g][:, sl])
            mm(psws[g][:, sl], WwT, sentsD[g][:, sl])
            nc.scalar.copy(out=Ws[g][:, sl], in_=psws[g][:, sl])

        def emit_chunk(g, c):
            # ksraw chunk: keys.s products for steps 4c..4c+3, reduced to
            # kg rows by two [1,320] f32r matmuls, then ekg = exp(-kg)
            ksr = work.tile([D, 4 * R], f32, tag="ksr", name=f"ksr{g}_{c}")
            kv = bass.AP(ksr.tensor, ksr.offset,
                         [list(ksr[:].ap[0]), [R, 4], [M, BL], [1, M]])
            s_sl = sentsD[g][:, 32 * c:32 * c + 32]
            s3 = bass.AP(s_sl.tensor, s_sl.offset,
                         [list(s_sl.ap[0]), [8, 4], [1, 8]])
            nc.vector.tensor_tensor(out=kv.bitcast(f32r),
                                    in0=bcast_mid2(keysT, 4, BL),
                                    in1=bcast_last3(s3, M), op=OP.mult)
            ksf = ksr[:]
            for h in range(2):
                pskg = ps_loop.tile([1, 2 * R], f32, tag="pskg",
                                    name=f"pskg{g}_{c}_{h}")
                mm(pskg[:], onesD, ksf[:, 2 * R * h:2 * R * (h + 1)])
                nc.scalar.activation(
                    out=ekg[g][:, 4 * c * R + 2 * R * h:4 * c * R + 2 * R * (h + 1)],
                    in_=pskg[:], func=AF.Exp, scale=-1.0)
            # vw chunk: Vk + Ws broadcast, [D, 4*160] view [D, 4, 8, 20]
            vsl = vw[g][:, 4 * c * R:(4 * c + 4) * R]
            out = bass.AP(vsl.tensor, vsl.offset,
                          [list(vsl.ap[0]), [R, 4], [M, BL], [1, M]])
            ws_sl = Ws[g][:, 32 * c:32 * c + 32]
            ws3 = bass.AP(ws_sl.tensor, ws_sl.offset,
                          [list(ws_sl.ap[0]), [8, 4], [1, 8]])
            nc.vector.tensor_tensor(out=out, in0=bcast_mid2(Vk[:], 4, BL),
                                    in1=bcast_last3(ws3, M), op=OP.add)

        # ---- lead-in: group 0 gathers + postproc + chunks; group 1 streams
        # during window 0 ----
        for c in range(GPC):
            emit_dma(c)
        for c in range(GPC):
            emit_mm1(c)
        for q in range(4):
            emit_qfinalize(0, q)
            emit_chunk(0, q)
        gcur = [GPC]                        # global gather cursor (group 0 done)

        # ---- initial state ----
        U = work.tile([D, R], f32, tag="U")
        nc.vector.tensor_copy(out=r_(U[:].rearrange("d (b m) -> d b m", m=M)),
                              in_=bcast_mid(keysT, BL))
        rho = None
        vkwsn = None

        # ---- recurrence ----
        for t in range(n_steps):
            g, k = t // GSTEP, t % GSTEP
            rs = (t % RESCALE == 0)           # U exactly normalized (rho=n=1)
            rescale_now = ((t + 1) % RESCALE == 0)

            # stream remaining gathers continuously (4 per step, global
            # cursor); quarter-finalize/chunks pipelined one window ahead
            for _ in range(4):
                if gcur[0] < NG * GPC:
                    emit_gather(gcur[0])
                    gcur[0] += 1
                elif gcur[0] == NG * GPC:
                    flush_mm1()
                    gcur[0] += 1
            gn = g + 1
            if gn < NG and k in (3, 5, 7, 9):
                q = (k - 3) // 2
                emit_qfinalize(gn, q)
                emit_chunk(gn, q)

            # cand (n-scaled): psA = Uw@U; candf = psA + vw*n
            psA = ps_loop.tile([D, R], f32, tag="psA")
            mm(psA[:], UwT, U[:])

            # gate reduce: psmg = 1^T (U . s_t)
            mgt = work.tile([D, BL, M], f32, tag="mgt")
            nc.vector.tensor_tensor(
                out=r_(mgt[:]),
                in0=U[:].rearrange("d (b m) -> d b m", m=M),
                in1=bcast_last(sentsD[g][:, BL * k:BL * (k + 1)], M),
                op=OP.mult)
            psmg = ps_loop.tile([1, R], f32, tag="psmg")
            mm(psmg[:], onesD, mgt[:].rearrange("d b m -> d (b m)"))

            if rs:
                l_ap = psmg[:]
            else:
                l_sb = work.tile([1, R], f32, tag="l")
                nc.vector.tensor_tensor(out=l_sb[:], in0=psmg[:], in1=rho[:],
                                        op=OP.mult)
                l_ap = l_sb[:]
            e1 = work.tile([1, R], f32, tag="e1")
            nc.scalar.activation(out=e1[:], in_=l_ap, func=AF.Exp, scale=-1.0)
            e_sb = work.tile([1, R], bf16, tag="e")
            nc.vector.tensor_tensor(out=e_sb[:], in0=e1[:],
                                    in1=ekg[g][:, k * R:(k + 1) * R], op=OP.mult)

            # candf = psA + vw_t * n  (vkwsn from prev tail; raw vw when n=1)
            candf = work.tile([D, R], f32, tag="candf")
            if rs:
                nc.vector.tensor_tensor(out=candf[:], in0=psA[:],
                                        in1=vw[g][:, k * R:(k + 1) * R],
                                        op=OP.add)
            else:
                nc.vector.tensor_tensor(out=candf[:], in0=psA[:],
                                        in1=vkwsn, op=OP.add)

            # U' = (1 + e) . U + candf
            psbce = ps_loop.tile([D, R], f32, tag="psbce")
            nc.tensor.matmul(out=psbce[:], lhsT=ones1b[:], rhs=e_sb[:],
                             start=True, stop=True)
            V_sb = work.tile([D, R], f32, tag="V")
            nc.vector.scalar_tensor_tensor(out=V_sb[:], in0=psbce[:],
                                           scalar=1.0, in1=U[:],
                                           op0=OP.add, op1=OP.mult)
            U2 = work.tile([D, R], f32, tag="U")
            nc.vector.tensor_tensor(out=r_(U2[:]), in0=V_sb[:], in1=candf[:],
                                    op=OP.add)
            U = U2

            # norms: rho' = exp(-0.5 ln ss), n' = ss * rho'
            sq = work.tile([D, R], f32, tag="sq")
            nc.vector.tensor_tensor(out=r_(sq[:]), in0=U[:], in1=U[:],
                                    op=OP.mult)
            psss = ps_loop.tile([1, R], f32, tag="psss")
            mm(psss[:], onesD, sq[:])
            lnss = work.tile([1, R], f32, tag="lnss")
            nc.scalar.activation(out=lnss[:], in_=psss[:], func=AF.Ln)
            rho2 = work.tile([1, R], f32, tag="rho")
            nc.scalar.activation(out=r_(rho2[:]), in_=lnss[:], func=AF.Exp,
                                 scale=-0.5)
            rho = rho2

            if rescale_now:
                # exact renormalization: U *= bc(rho); afterwards rho = n = 1
                psbcr = ps_loop.tile([D, R], f32, tag="psbcn", name=f"psbcr{t}")
                mm(psbcr[:], ones1, rho[:])
                U3 = work.tile([D, R], f32, tag="U")
                nc.vector.tensor_tensor(out=r_(U3[:]), in0=psbcr[:], in1=U[:],
                                        op=OP.mult)
                U = U3
            elif t < n_steps - 1:
                # n' for next step's vkwsn
                n_sb = work.tile([1, R], bf16, tag="n")
                nc.vector.tensor_tensor(out=n_sb[:], in0=psss[:], in1=rho[:],
                                        op=OP.mult)
                psbcn = ps_loop.tile([D, R], f32, tag="psbcn")
                nc.tensor.matmul(out=psbcn[:], lhsT=ones1b[:], rhs=n_sb[:],
                                 start=True, stop=True)
                tn = t + 1
                gg, kk = tn // GSTEP, tn % GSTEP
                vkwsn2 = work.tile([D, R], f32, tag="vkwsn")
                nc.vector.tensor_tensor(out=vkwsn2[:], in0=psbcn[:],
                                        in1=vw[gg][:, kk * R:(kk + 1) * R],
                                        op=OP.mult)
                vkwsn = vkwsn2[:]

        # ---- output ----
        if n_steps % RESCALE == 0:
            nc.sync.dma_start(out=out_d, in_=U[:])
        else:
            psbcr = ps_loop.tile([D, R], f32, tag="psbcn")
            mm(psbcr[:], ones1, rho[:])
            memT = work.tile([D, R], f32, tag="memT")
            nc.vector.tensor_tensor(out=memT[:], in0=psbcr[:], in1=U[:],
                                    op=OP.mult)
            nc.sync.dma_start(out=out_d, in_=memT[:])

    _strip_redundant_self_waits(nc)
    return nc


def _np_fallback(tokens, emb, keys, mult, Uw, Vw, Ww, prelu_a):
    tokens = np.asarray(tokens)
    emb = np.asarray(emb, np.float32)
    keys = np.asarray(keys, np.float32)
    mult = np.asarray(mult, np.float32)
    Uw, Vw, Ww = (np.asarray(x, np.float32) for x in (Uw, Vw, Ww))
    a = float(np.asarray(prelu_a).reshape(-1)[0])
    x = emb[tokens] * mult
    sents = x.sum(axis=2, dtype=np.float32)
    mem = np.broadcast_to(keys, (tokens.shape[0], M, D)).astype(np.float32).copy()
    Vk = keys @ Vw.T
    for t in range(tokens.shape[1]):
        s = sents[:, t, :]
        logits = np.einsum('bd,bmd->bm', s, mem) + s @ keys.T
        gate = 1.0 / (1.0 + np.exp(-logits))
        pre = mem @ Uw.T + Vk + (s @ Ww.T)[:, None, :]
        cand = np.where(pre >= 0, pre, a * pre)
        mask = np.where(logits == 0.0, 0.0, 1.0)
        mem = mem + cand * (gate * mask)[:, :, None]
        mem = mem / np.linalg.norm(mem, axis=2, keepdims=True)
    return mem.astype(np.float32)


def _stage_inputs(tokens, emb, keys, mult, Uw, Vw, Ww, prelu_a):
    """Host-side sharding/layout prep. Returns (in_maps, flags)."""
    tokens = np.asarray(tokens)
    emb = np.asarray(emb, dtype=np.float32)
    keys = np.asarray(keys, dtype=np.float32)
    mult = np.asarray(mult, dtype=np.float32)
    a = float(np.asarray(prelu_a).reshape(-1)[0])
    a_is_one = (a == 1.0)
    mult_is_ones = bool(np.all(mult == 1.0))

    emb = np.ascontiguousarray(emb)

    CW = 549
    consts = np.zeros((128, CW), np.float32)
    consts[0:D, 0:M] = keys.T
    consts[0:D, 20:120] = np.asarray(Uw, np.float32).T        # lhsT for Uw@mem
    consts[0:D, 120:220] = np.asarray(Ww, np.float32).T
    consts[0:D, 220:320] = np.asarray(Vw, np.float32).T
    consts[0:D, 320:321] = 1.0                                # onesD
    consts[0:1, 321:421] = 1.0                                # ones1
    consts[0:128, 421:425] = np.kron(np.eye(4, dtype=np.float32),
                                     np.ones((L, 1), np.float32))  # blk

    in_maps = []
    for c in range(NCORES):
        tc_ = tokens[c * BL:(c + 1) * BL]                     # [8, S, L]
        # sentence-major rows with t-major sentence order: row j = 8t+b
        tokflat = np.ascontiguousarray(tc_.transpose(1, 0, 2)).reshape(NS, L)
        # tok_staged[p, c] = token of sentence 4c + p//32, word p%32
        tok_staged = np.ascontiguousarray(
            tokflat.reshape(2 * S, 4, L).transpose(1, 2, 0)).reshape(128, 2 * S)
        in_maps.append({"tok": np.ascontiguousarray(tok_staged, np.int32),
                        "emb": emb, "consts": consts})
    return in_maps, a_is_one, mult_is_ones, a


def kernel(tokens, emb, keys, mult, Uw, Vw, Ww, prelu_a, _trace=False):
    from concourse.bass_utils import run_bass_kernel_spmd

    in_maps, a_is_one, mult_is_ones, a = _stage_inputs(
        tokens, emb, keys, mult, Uw, Vw, Ww, prelu_a)
    if not (a_is_one and mult_is_ones):
        return _np_fallback(tokens, emb, keys, mult, Uw, Vw, Ww, prelu_a)

    key = (a_is_one, mult_is_ones)
    if key not in _prog_cache:
        _prog_cache[key] = _build_program(a_is_one, mult_is_ones, a)
    nc = _prog_cache[key]

    res = run_bass_kernel_spmd(nc, in_maps, list(range(NCORES)), trace=_trace)
    out = np.empty((B, M, D), dtype=np.float32)
    for c in range(NCORES):
        memT = res.results[c]["memT"]                          # [D, R]
        out[c * BL:(c + 1) * BL] = memT.reshape(D, BL, M).transpose(1, 2, 0)
    kernel._last_results = res
    return out


# revision 16
# speedup vs baseline: 1.3183x; 1.0010x over previous
"""EntNet Trainium2 kernel (8-core data-parallel over batch).

Reference computation (shapes: B=64, S=128, L=32, D=100, M=20, V=50000):
  sents = (emb[tokens] * mult).sum(axis=2)            # [B,S,D]
  mem0 = broadcast(keys)                              # [B,M,D]
  per step t: gate = sigmoid(s.mem + s.keys); cand = prelu(mem@Uw.T + keys@Vw.T + s@Ww.T)
              mem = normalize(mem + cand*gate, axis=D)

Kernel strategy per core (8 batches/core, R = 8*20 = 160 (b,m) rows):
  - Embedding gather via gpsimd indirect DMA (128 tokens = 4 sentences per
    instruction), word-summed by one tiny f32r matmul per gather straight
    into a per-group D-major psum [100,128] (Tile's scheduler models bypass
    gathers accurately, so it interleaves the stream without queue stalls;
    accumulating gathers run 60% over model and head-of-line block queues).
  - Key-gate logits kg = s.keys precomputed per group from sents via
    keys-broadcast DVE products reduced by [1,320] f32r matmuls.
  - Recurrence kept in scale-free form: with U unnormalized and
    rho = 1/||U|| per row, the update
        mem' = normalize(mem + cand * sigmoid(l))
    is exactly
        U' = (1 + exp(-l)) . U + Uw@U + (Vk + Ws_t) * n        (n = ||U||)
    Gate split: exp(-l) = exp(-rho * 1^T(U.s)) * exp(-kg), with exp(-kg)
    precomputed per 16-step group (kg rows extracted by a SBUF->SBUF DMA
    transpose-gather), so no key-term matmul and no n-scaled keys on the
    critical path.
  - All matmuls run as float32r (single-pass PE mode) instead of float32
    (which costs 2 half-rate passes + 2 LDWEIGHTS each).
"""

import numpy as np

B, S, L, D, M, V = 64, 128, 32, 100, 20, 50000
NCORES = 8
BL = B // NCORES            # 8 batches per core
NS = BL * S                 # 1024 sentences per core
R = BL * M                  # 160 (b, m) rows per core
NG = 8                      # gather groups (16 steps each)
GSTEP = S // NG             # 16 steps per group
GPC = 32                    # gathers per group (4 sentences each)
RESCALE = 8                 # renormalize U every RESCALE steps (f32 range)

_prog_cache = {}

_ENGINE_SEM = {"PE": "PE_", "DVE": "DVE_", "Activation": "Activation_",
               "Pool": "Pool_", "SP": "SP_"}


def _strip_redundant_self_waits(nc):
    """Legalize sync waits: walrus rejects >1 sync wait on most instruction
    structs. For any instruction carrying several, hoist all but one onto
    preceding single-wait NoOps on the same engine queue (in-order dispatch
    keeps semantics). The instruction keeps its OWN-engine wait if it has one
    (that wait guards an engine-pipelining RAW hazard and must gate execution,
    not just dispatch).
    """
    import concourse.mybir as mybir
    for fn in nc.m.functions:
        for blk in fn.blocks:
            i = 0
            while i < len(blk.instructions):
                inst = blk.instructions[i]
                si = inst.sync_info() if callable(inst.sync_info) else inst.sync_info
                if (si is not None and si.on_wait and len(si.on_wait) > 1
                        and inst.engine is not None):
                    waits = list(si.on_wait)
                    pref = _ENGINE_SEM.get(inst.engine.name)
                    keep_idx = None
                    for j, w in enumerate(waits):
                        if pref and w.ant_name.startswith(pref):
                            keep_idx = j
                            break
                    kept = [waits.pop(keep_idx)] if keep_idx is not None else []
                    noops = []
                    for w in waits:
                        nop = mybir.InstNoOp(
                            name=nc.get_next_instruction_name(), ins=[], outs=[])
                        nop.engine = inst.engine
                        nop.sync_info = mybir.SyncInfo(on_wait=[w], on_update=[])
                        nc.register_instruction(nop, overwrite=True)
                        noops.append(nop)
                    inst.sync_info = mybir.SyncInfo(
                        on_wait=kept, on_update=list(si.on_update))
                    blk.instructions[i:i] = noops
                    i += len(noops)
                i += 1


def _build_program(a_is_one: bool, mult_is_ones: bool, alpha: float,
                   n_steps: int = S, dump: bool = False):
    import concourse.bass as bass
    import concourse.tile as tile
    from concourse import mybir
    from contextlib import ExitStack

    assert mult_is_ones and a_is_one, "fast path only (host fallback otherwise)"

    f32 = mybir.dt.float32
    f32r = mybir.dt.float32r
    i32 = mybir.dt.int32
    AF = mybir.ActivationFunctionType
    OP = mybir.AluOpType

    nc = bass.Bass(trn_type="TRN2")

    # ---- DRAM I/O ----
    CW = 549
    tok_d = nc.dram_tensor("tok", [128, NG * L], i32, kind="ExternalInput").ap()
    emb_d = nc.dram_tensor("emb", [V, D], f32r, kind="ExternalInput").ap()
    consts_d = nc.dram_tensor("consts", [128, CW], f32r, kind="ExternalInput").ap()
    out_d = nc.dram_tensor("memT", [D, R], f32, kind="ExternalOutput").ap()

    def r_(ap):
        # f32r = fp32 bits fed to the PE in single-pass (replicated) mode.
        # walrus's BIR verifier requires producers of f32r-matmul operands to
        # declare f32r-rounded output, so producer OUT aps are bitcast too.
        return ap.bitcast(f32r)

    def mm(out, lhsT, rhs, start=True, stop=True):
        nc.tensor.matmul(out=out, lhsT=r_(lhsT), rhs=r_(rhs),
                         start=start, stop=stop)

    def bcast_mid(ap_2d, n_mid):
        # [P, k] -> [P, n_mid, k] with stride-0 middle dim
        return bass.AP(ap_2d.tensor, ap_2d.offset,
                       [list(ap_2d.ap[0]), [0, n_mid], list(ap_2d.ap[1])])

    def bcast_last(ap_2d, n_last):
        # [P, k] -> [P, k, n_last] with stride-0 last dim
        return bass.AP(ap_2d.tensor, ap_2d.offset,
                       [list(ap_2d.ap[0]), list(ap_2d.ap[1]), [0, n_last]])

    def bcast_mid2(ap_2d, n1, n2):
        # [P, k] -> [P, n1, n2, k] with stride-0 middle dims
        return bass.AP(ap_2d.tensor, ap_2d.offset,
                       [list(ap_2d.ap[0]), [0, n1], [0, n2], list(ap_2d.ap[1])])

    def bcast_last3(ap_3d, n_last):
        # [P, a, b] -> [P, a, b, n_last] with stride-0 last dim
        return bass.AP(ap_3d.tensor, ap_3d.offset,
                       [list(ap_3d.ap[0]), list(ap_3d.ap[1]),
                        list(ap_3d.ap[2]), [0, n_last]])

    with tile.TileContext(nc) as tc, ExitStack() as ctx:
        const = ctx.enter_context(tc.tile_pool(name="const", bufs=1))
        work = ctx.enter_context(tc.tile_pool(name="work", bufs=2))
        ps_setup = ctx.enter_context(tc.tile_pool(name="ps_setup", bufs=1, space="PSUM"))
        ps_loop = ctx.enter_context(tc.tile_pool(name="ps_loop", bufs=1, space="PSUM"))

        # ---- load constants / weights ----
        tok_sb = const.tile([128, NG * L], i32)
        nc.gpsimd.dma_start(out=tok_sb[:], in_=tok_d)
        consts = const.tile([128, CW], f32)
        nc.sync.dma_start(out=r_(consts[:]), in_=consts_d)
        keysT = consts[0:D, 0:M]
        UwT = consts[0:D, 20:120]
        WwT = consts[0:D, 120:220]
        VwT = consts[0:D, 220:320]
        onesD = consts[0:D, 320:321]
        ones1 = consts[0:1, 321:421]
        blk = consts[0:128, 421:425]

        # ---- Vk = Vw @ keys^T (early; only needs weights) ----
        ps_vk = ps_setup.tile([D, M], f32, tag="psws", name="ps_vk")
        mm(ps_vk[:], VwT, keysT)
        Vk = const.tile([D, M], f32)
        nc.scalar.copy(out=Vk[:], in_=ps_vk[:])
        # bf16 ones row for the cheap 1-cycle/col broadcast matmuls
        bf16 = mybir.dt.bfloat16
        ones1b = const.tile([1, 100], bf16)
        nc.scalar.copy(out=ones1b[:], in_=ones1)

        # ---- gather machinery ----
        # group g covers steps 16g..16g+15 = sentences 128g..128g+127
        # (t-major: sentence j = 8t+b). gather c covers sentences 4c..4c+3:
        # 128 tokens, one per partition; word-sum via f32r matmul with the
        # block-ones matrix into the group psum (D-major, cols = sentence).
        gpool = ctx.enter_context(tc.tile_pool(name="gath", bufs=64))
        sentsD = [const.tile([D, 128], f32, name=f"sentsD{g}", tag=f"sD{g % 2}")
                  for g in range(NG)]
        Ws = [const.tile([D, 128], f32, name=f"ws{g}", tag=f"ws{g % 2}")
              for g in range(NG)]
        ekg = [const.tile([1, GSTEP * R], f32, name=f"ekg{g}",
                          tag=f"ekg{g % 2}") for g in range(NG)]
        vw = [const.tile([D, GSTEP * R], f32, name=f"vw{g}", tag=f"vw{g % 2}")
              for g in range(NG)]
        ps_grp = [ps_setup.tile([D, 128], f32, tag="psgrp", name=f"psgrp{g}")
                  for g in range(NG)]

        gtiles = {}

        def emit_dma(c):
            gt = gpool.tile([128, D], f32, tag="g", name=f"g{c}")
            gtiles[c] = gt
            with tc.high_priority():
                nc.gpsimd.indirect_dma_start(
                    out=r_(gt[:]),
                    out_offset=None,
                    in_=emb_d,
                    in_offset=bass.IndirectOffsetOnAxis(
                        ap=tok_sb[:, c:c + 1], axis=0),
                )

        def emit_mm1(c):
            # priority 0: the scheduler slots each reduce-matmul into the
            # first PE gap after its gather lands instead of batch-deferring
            # the backlog into window-boundary gaps (10us+ real stalls).
            g = c // GPC
            j = c % GPC
            gt = gtiles.pop(c)
            with tc.high_priority():
                mm(ps_grp[g][:, 4 * j:4 * j + 4], gt[:], blk)

        MMLAG = 8

        def emit_gather(c):
            # DMA for gather c; reduce-matmul for gather c-MMLAG. The lag
            # keeps per-gather matmuls queued on PE well after their DMA has
            # landed, so the in-order PE queue never blocks on the Pool-bound
            # gather stream.
            emit_dma(c)
            if c - MMLAG >= GPC:
                emit_mm1(c - MMLAG)

        def flush_mm1():
            for c in range(NG * GPC - MMLAG, NG * GPC):
                emit_mm1(c)

        psws = [ps_setup.tile([D, 128], f32, tag="psws", name=f"psws{g}")
                for g in range(NG)]

        def emit_qfinalize(g, q):
            # quarter-granular: only depends on gathers 8q..8q+8 of group g
            sl = slice(32 * q, 32 * q + 32)
            nc.scalar.copy(out=r_(sentsD[g][:, sl]), in_=ps_grp[g][:, sl])
            mm(psws[g][:, sl], WwT, sentsD[g][:, sl])
            nc.scalar.copy(out=Ws[g][:, sl], in_=psws[g][:, sl])

        def emit_chunk(g, c):
            # ksraw chunk: keys.s products for steps 4c..4c+3, reduced to
            # kg rows by two [1,320] f32r matmuls, then ekg = exp(-kg)
            ksr = work.tile([D, 4 * R], f32, tag="ksr", name=f"ksr{g}_{c}")
            kv = bass.AP(ksr.tensor, ksr.offset,
                         [list(ksr[:].ap[0]), [R, 4], [M, BL], [1, M]])
            s_sl = sentsD[g][:, 32 * c:32 * c + 32]
            s3 = bass.AP(s_sl.tensor, s_sl.offset,
                         [list(s_sl.ap[0]), [8, 4], [1, 8]])
            nc.vector.tensor_tensor(out=kv.bitcast(f32r),
                                    in0=bcast_mid2(keysT, 4, BL),
                                    in1=bcast_last3(s3, M), op=OP.mult)
            ksf = ksr[:]
            for h in range(2):
                pskg = ps_loop.tile([1, 2 * R], f32, tag="pskg",
                                    name=f"pskg{g}_{c}_{h}")
                mm(pskg[:], onesD, ksf[:, 2 * R * h:2 * R * (h + 1)])
                nc.scalar.activation(
                    out=ekg[g][:, 4 * c * R + 2 * R * h:4 * c * R + 2 * R * (h + 1)],
                    in_=pskg[:], func=AF.Exp, scale=-1.0)
            # vw chunk: Vk + Ws broadcast, [D, 4*160] view [D, 4, 8, 20]
            vsl = vw[g][:, 4 * c * R:(4 * c + 4) * R]
            out = bass.AP(vsl.tensor, vsl.offset,
                          [list(vsl.ap[0]), [R, 4], [M, BL], [1, M]])
            ws_sl = Ws[g][:, 32 * c:32 * c + 32]
            ws3 = bass.AP(ws_sl.tensor, ws_sl.offset,
                          [list(ws_sl.ap[0]), [8, 4], [1, 8]])
            nc.vector.tensor_tensor(out=out, in0=bcast_mid2(Vk[:], 4, BL),
                                    in1=bcast_last3(ws3, M), op=OP.add)

        # ---- lead-in: group 0 gathers + postproc + chunks; group 1 streams
        # during window 0 ----
        for c in range(GPC):
            emit_dma(c)
        for c in range(GPC):
            emit_mm1(c)
        for q in range(4):
            emit_qfinalize(0, q)
            emit_chunk(0, q)
        gcur = [GPC]                        # global gather cursor (group 0 done)

        # ---- initial state ----
        U = work.tile([D, R], f32, tag="U")
        nc.vector.tensor_copy(out=r_(U[:].rearrange("d (b m) -> d b m", m=M)),
                              in_=bcast_mid(keysT, BL))
        rho = None
        vkwsn = None

        # ---- recurrence ----
        for t in range(n_steps):
            g, k = t // GSTEP, t % GSTEP
            rs = (t % RESCALE == 0)           # U exactly normalized (rho=n=1)
            rescale_now = ((t + 1) % RESCALE == 0)

            # stream remaining gathers continuously (4 per step, global
            # cursor); quarter-finalize/chunks pipelined one window ahead
            for _ in range(4):
                if gcur[0] < NG * GPC:
                    emit_gather(gcur[0])
                    gcur[0] += 1
                elif gcur[0] == NG * GPC:
                    flush_mm1()
                    gcur[0] += 1
            gn = g + 1
            if gn < NG and k in (3, 5, 7, 9):
                q = (k - 3) // 2
                emit_qfinalize(gn, q)
                emit_chunk(gn, q)

            # cand (n-scaled): psA = Uw@U; candf = psA + vw*n
            psA = ps_loop.tile([D, R], f32, tag="psA")
            mm(psA[:], UwT, U[:])

            # gate reduce: psmg = 1^T (U . s_t)
            mgt = work.tile([D, BL, M], f32, tag="mgt")
            nc.vector.tensor_tensor(
                out=r_(mgt[:]),
                in0=U[:].rearrange("d (b m) -> d b m", m=M),
                in1=bcast_last(sentsD[g][:, BL * k:BL * (k + 1)], M),
                op=OP.mult)
            psmg = ps_loop.tile([1, R], f32, tag="psmg")
            mm(psmg[:], onesD, mgt[:].rearrange("d b m -> d (b m)"))

            if rs:
                l_ap = psmg[:]
            else:
                l_sb = work.tile([1, R], f32, tag="l")
                nc.vector.tensor_tensor(out=l_sb[:], in0=psmg[:], in1=rho[:],
                                        op=OP.mult)
                l_ap = l_sb[:]
            e1 = work.tile([1, R], f32, tag="e1")
            nc.scalar.activation(out=e1[:], in_=l_ap, func=AF.Exp, scale=-1.0)
            e_sb = work.tile([1, R], bf16, tag="e")
            nc.vector.tensor_tensor(out=e_sb[:], in0=e1[:],
                                    in1=ekg[g][:, k * R:(k + 1) * R], op=OP.mult)

            # candf = psA + vw_t * n  (vkwsn from prev tail; raw vw when n=1)
            candf = work.tile([D, R], f32, tag="candf")
            if rs:
                nc.vector.tensor_tensor(out=candf[:], in0=psA[:],
                                        in1=vw[g][:, k * R:(k + 1) * R],
                                        op=OP.add)
            else:
                nc.vector.tensor_tensor(out=candf[:], in0=psA[:],
                                        in1=vkwsn, op=OP.add)

            # U' = (1 + e) . U + candf
            psbce = ps_loop.tile([D, R], f32, tag="psbce")
            nc.tensor.matmul(out=psbce[:], lhsT=ones1b[:], rhs=e_sb[:],
                             start=True, stop=True)
            V_sb = work.tile([D, R], f32, tag="V")
            nc.vector.scalar_tensor_tensor(out=V_sb[:], in0=psbce[:],
                                           scalar=1.0, in1=U[:],
                                           op0=OP.add, op1=OP.mult)
            U2 = work.tile([D, R], f32, tag="U")
            nc.vector.tensor_tensor(out=r_(U2[:]), in0=V_sb[:], in1=candf[:],
                                    op=OP.add)
            U = U2

            # norms: rho' = exp(-0.5 ln ss), n' = ss * rho'
            sq = work.tile([D, R], f32, tag="sq")
            nc.vector.tensor_tensor(out=r_(sq[:]), in0=U[:], in1=U[:],
                                    op=OP.mult)
            psss = ps_loop.tile([1, R], f32, tag="psss")
            mm(psss[:], onesD, sq[:])
            lnss = work.tile([1, R], f32, tag="lnss")
            nc.scalar.activation(out=lnss[:], in_=psss[:], func=AF.Ln)
            rho2 = work.tile([1, R], f32, tag="rho")
            nc.scalar.activation(out=r_(rho2[:]), in_=lnss[:], func=AF.Exp,
                                 scale=-0.5)
            rho = rho2

            if rescale_now:
                # exact renormalization: U *= bc(rho); afterwards rho = n = 1
                psbcr = ps_loop.tile([D, R], f32, tag="psbcn", name=f"psbcr{t}")
                mm(psbcr[:], ones1, rho[:])
                U3 = work.tile([D, R], f32, tag="U")
                nc.vector.tensor_tensor(out=r_(U3[:]), in0=psbcr[:], in1=U[:],
                                        op=OP.mult)
                U = U3
            elif t < n_steps - 1:
                # n' for next step's vkwsn
                n_sb = work.tile([1, R], bf16, tag="n")
                nc.vector.tensor_tensor(out=n_sb[:], in0=psss[:], in1=rho[:],
                                        op=OP.mult)
                psbcn = ps_loop.tile([D, R], f32, tag="psbcn")
                nc.tensor.matmul(out=psbcn[:], lhsT=ones1b[:], rhs=n_sb[:],
                                 start=True, stop=True)
                tn = t + 1
                gg, kk = tn // GSTEP, tn % GSTEP
                vkwsn2 = work.tile([D, R], f32, tag="vkwsn")
                nc.vector.tensor_tensor(out=vkwsn2[:], in0=psbcn[:],
                                        in1=vw[gg][:, kk * R:(kk + 1) * R],
                                        op=OP.mult)
                vkwsn = vkwsn2[:]

        # ---- output ----
        if n_steps % RESCALE == 0:
            nc.sync.dma_start(out=out_d, in_=U[:])
        else:
            psbcr = ps_loop.tile([D, R], f32, tag="psbcn")
            mm(psbcr[:], ones1, rho[:])
            memT = work.tile([D, R], f32, tag="memT")
            nc.vector.tensor_tensor(out=memT[:], in0=psbcr[:], in1=U[:],
                                    op=OP.mult)
            nc.sync.dma_start(out=out_d, in_=memT[:])

    _strip_redundant_self_waits(nc)
    return nc


def _np_fallback(tokens, emb, keys, mult, Uw, Vw, Ww, prelu_a):
    tokens = np.asarray(tokens)
    emb = np.asarray(emb, np.float32)
    keys = np.asarray(keys, np.float32)
    mult = np.asarray(mult, np.float32)
    Uw, Vw, Ww = (np.asarray(x, np.float32) for x in (Uw, Vw, Ww))
    a = float(np.asarray(prelu_a).reshape(-1)[0])
    x = emb[tokens] * mult
    sents = x.sum(axis=2, dtype=np.float32)
    mem = np.broadcast_to(keys, (tokens.shape[0], M, D)).astype(np.float32).copy()
    Vk = keys @ Vw.T
    for t in range(tokens.shape[1]):
        s = sents[:, t, :]
        logits = np.einsum('bd,bmd->bm', s, mem) + s @ keys.T
        gate = 1.0 / (1.0 + np.exp(-logits))
        pre = mem @ Uw.T + Vk + (s @ Ww.T)[:, None, :]
        cand = np.where(pre >= 0, pre, a * pre)
        mask = np.where(logits == 0.0, 0.0, 1.0)
        mem = mem + cand * (gate * mask)[:, :, None]
        mem = mem / np.linalg.norm(mem, axis=2, keepdims=True)
    return mem.astype(np.float32)


def _stage_inputs(tokens, emb, keys, mult, Uw, Vw, Ww, prelu_a):
    """Host-side sharding/layout prep. Returns (in_maps, flags)."""
    tokens = np.asarray(tokens)
    emb = np.asarray(emb, dtype=np.float32)
    keys = np.asarray(keys, dtype=np.float32)
    mult = np.asarray(mult, dtype=np.float32)
    a = float(np.asarray(prelu_a).reshape(-1)[0])
    a_is_one = (a == 1.0)
    mult_is_ones = bool(np.all(mult == 1.0))

    emb = np.ascontiguousarray(emb)

    CW = 549
    consts = np.zeros((128, CW), np.float32)
    consts[0:D, 0:M] = keys.T
    consts[0:D, 20:120] = np.asarray(Uw, np.float32).T        # lhsT for Uw@mem
    consts[0:D, 120:220] = np.asarray(Ww, np.float32).T
    consts[0:D, 220:320] = np.asarray(Vw, np.float32).T
    consts[0:D, 320:321] = 1.0                                # onesD
    consts[0:1, 321:421] = 1.0                                # ones1
    consts[0:128, 421:425] = np.kron(np.eye(4, dtype=np.float32),
                                     np.ones((L, 1), np.float32))  # blk

    in_maps = []
    for c in range(NCORES):
        tc_ = tokens[c * BL:(c + 1) * BL]                     # [8, S, L]
        # sentence-major rows with t-major sentence order: row j = 8t+b
        tokflat = np.ascontiguousarray(tc_.transpose(1, 0, 2)).reshape(NS, L)
        # tok_staged[p, c] = token of sentence 4c + p//32, word p%32
        tok_staged = np.ascontiguousarray(
            tokflat.reshape(2 * S, 4, L).transpose(1, 2, 0)).reshape(128, 2 * S)
        in_maps.append({"tok": np.ascontiguousarray(tok_staged, np.int32),
                        "emb": emb, "consts": consts})
    return in_maps, a_is_one, mult_is_ones, a


def kernel(tokens, emb, keys, mult, Uw, Vw, Ww, prelu_a, _trace=False):
    from concourse.bass_utils import run_bass_kernel_spmd

    in_maps, a_is_one, mult_is_ones, a = _stage_inputs(
        tokens, emb, keys, mult, Uw, Vw, Ww, prelu_a)
    if not (a_is_one and mult_is_ones):
        return _np_fallback(tokens, emb, keys, mult, Uw, Vw, Ww, prelu_a)

    key = (a_is_one, mult_is_ones)
    if key not in _prog_cache:
        _prog_cache[key] = _build_program(a_is_one, mult_is_ones, a)
    nc = _prog_cache[key]

    res = run_bass_kernel_spmd(nc, in_maps, list(range(NCORES)), trace=_trace)
    out = np.empty((B, M, D), dtype=np.float32)
    for c in range(NCORES):
        memT = res.results[c]["memT"]                          # [D, R]
        out[c * BL:(c + 1) * BL] = memT.reshape(D, BL, M).transpose(1, 2, 0)
    kernel._last_results = res
    return out
